# revision 2
# baseline (speedup 1.0000x reference)
"""DNC sequential kernel v2 for Trainium2 — row-major redesign.

Single-core Bass/Tile implementation of the 8192-step DNC recurrence.
Design vs the v1 baseline:
- Replicated-output PE matmuls give the interface activations pre-broadcast
  over the 10 memory cells; no gpsimd partition_broadcast anywhere.
- stream_shuffle (DVE) does the remaining partition broadcasts.
- All dot products via scalar_tensor_tensor + accum_out (1 DVE op) or tiny
  PE matmuls in row form.
- Sort-free allocation: pairwise-compare matrix + product-reduce (DVE
  tensor_reduce with mult), stable-argsort tie semantics via is_equal*JLT.
- Read vector rv is never materialized: gates0 use W0rM = M @ W_ih0[:,10:]^T
  (f32r matmul, precomputed per step once M is updated) applied to rw, and
  the output projection uses WcM = M @ Wc2^T applied to rw.
- ACT table set 6 (exp/ln) preloaded once; sigmoid/tanh via exp with signs
  folded into host weights; softplus = Ln(1+e^x) exact; sqrt via Ln+Exp(0.5).
- All biases in this problem are zero (asserted on host) and dropped.
"""

import sys
import os
import numpy as np

if "/opt/trn_rl_repo" not in sys.path:
    sys.path.insert(0, "/opt/trn_rl_repo")

N, CELL, R, H, X_DIM, OUT_DIM = 10, 20, 1, 128, 10, 10
EPS, DELTA = 1e-6, 1e-6
SEQ_LEN = 8192


def build(T=SEQ_LEN, U=8):
    import concourse.bass as bass
    import concourse.bacc as bacc
    import concourse.mybir as mybir
    from concourse import tile

    F32 = mybir.dt.float32
    F32R = mybir.dt.float32r
    F16 = mybir.dt.float16
    AF = mybir.ActivationFunctionType
    OP = mybir.AluOpType
    AX = mybir.AxisListType
    ds = bass.ds

    assert T % U == 0

    nc = bacc.Bacc(None, target_bir_lowering=False, debug=False)
    names = {}

    with tile.TileContext(nc) as tc:
        with tc.tile_pool(name="dram", bufs=1, space="DRAM") as dram, \
             tc.tile_pool(name="consts", bufs=1) as consts, \
             tc.tile_pool(name="state", bufs=1) as state, \
             tc.tile_pool(name="work", bufs=2) as work, \
             tc.tile_pool(name="io", bufs=2) as io, \
             tc.tile_pool(name="psG", bufs=2, space="PSUM") as psG, \
             tc.tile_pool(name="psA", bufs=1, space="PSUM") as psA, \
             tc.tile_pool(name="psB", bufs=1, space="PSUM") as psB, \
             tc.tile_pool(name="psW", bufs=1, space="PSUM") as psW:

            # ---------------- DRAM I/O ----------------
            d_xb3 = dram.tile([128, T // U, 4 * U], F32, kind="ExternalInput")
            d_w0ht = dram.tile([128, 512], F16, kind="ExternalInput")
            d_w0rmt = dram.tile([20, 512], F32R, kind="ExternalInput")
            d_w1it = dram.tile([128, 512], F16, kind="ExternalInput")
            d_w1ht = dram.tile([128, 512], F16, kind="ExternalInput")
            d_wxit = dram.tile([128, 58], F16, kind="ExternalInput")
            d_wrkt = dram.tile([128, 20], F16, kind="ExternalInput")
            d_wwkt = dram.tile([128, 20], F16, kind="ExternalInput")
            d_wct1 = dram.tile([128, 10], F16, kind="ExternalInput")
            d_wct2 = dram.tile([20, 10], F32, kind="ExternalInput")
            d_jlt = dram.tile([10, 10], F32, kind="ExternalInput")
            d_diagm = dram.tile([10, 10], F32, kind="ExternalInput")
            d_yt = dram.tile([10, T // U, U], F32, kind="ExternalOutput")
            for k, v in [("xb3", d_xb3), ("w0ht", d_w0ht), ("w0rmt", d_w0rmt),
                         ("w1it", d_w1it), ("w1ht", d_w1ht), ("wxit", d_wxit),
                         ("wrkt", d_wrkt), ("wwkt", d_wwkt), ("wct1", d_wct1),
                         ("wct2", d_wct2),
                         ("jlt", d_jlt), ("diagm", d_diagm),
                         ("yt", d_yt)]:
                names[k] = v.tensor.name

            # preload ACT set 6 (exp/ln) once; all activations stay in-set
            nc.scalar.add_instruction(mybir.InstLoadActFuncSet(
                name=nc.get_next_instruction_name(), act_func_set_id=6,
                ins=[], outs=[]))

            # ---------------- const SBUF ----------------
            W0HT = consts.tile([128, 512], F16)
            W0RMT = consts.tile([20, 512], F32R)
            W1IT = consts.tile([128, 512], F16)
            W1HT = consts.tile([128, 512], F16)
            WXIT = consts.tile([128, 58], F16)
            WRKT = consts.tile([128, 20], F16)
            WWKT = consts.tile([128, 20], F16)
            WCT1 = consts.tile([128, 10], F16)
            WCT2 = consts.tile([20, 10], F32)
            JLT = consts.tile([10, 10], F32)
            DIAGM = consts.tile([10, 10], F32)
            E38 = consts.tile([1, 10], F32)
            ONESC = consts.tile([10, 1], F32)
            ONES10 = consts.tile([128, 10], F32)

            for dst, src in [(W0HT, d_w0ht), (W0RMT, d_w0rmt), (W1IT, d_w1it),
                             (W1HT, d_w1ht), (WXIT, d_wxit), (WRKT, d_wrkt),
                             (WWKT, d_wwkt), (WCT1, d_wct1), (WCT2, d_wct2),
                             (JLT, d_jlt), (DIAGM, d_diagm)]:
                nc.sync.dma_start(dst[:], src[:])
            nc.vector.memset(E38[:], 1e-36)
            nc.vector.memset(ONESC[:], 1.0)
            nc.vector.memset(ONES10[:], 1.0)

            # ---------------- state SBUF ----------------
            h0 = state.tile([128, 1], F16)
            c0 = state.tile([128, 1], F32)
            h1rep = state.tile([128, 10], F16)   # h1 replicated; col 0 = h1
            c1 = state.tile([128, 1], F32)
            M32 = state.tile([32, 32], F32)      # M  = [0:10, 0:20]
            Mt32 = state.tile([32, 32], F32)     # Mt = [0:20, 0:10]
            L32 = state.tile([32, 32], F32)      # L  = [0:10, 0:10]
            LT32 = state.tile([32, 32], F32)
            Mtr = state.tile([20, 10], F32R)     # f32r copy of Mt
            zM32 = state.tile([32, 32], F32)     # col 0 = zM (row norms^2)
            zMr32 = state.tile([32, 32], F32)    # row 0 = zM as a row
            RW32 = state.tile([32, 32], F32)     # row 0 = rw_row
            RWT32 = state.tile([32, 32], F32)    # col 0 = rw_col
            WW32 = state.tile([32, 32], F32)     # row 0 = ww_row
            WWT32 = state.tile([32, 32], F32)    # col 0 = ww_col
            Psh = state.tile([32, 32], F32)      # row 0 = p_row
            P10b = state.tile([32, 10], F32)     # p broadcast
            p10m = state.tile([10, 10], F32)     # p bcast, diag-masked
            UEC32 = state.tile([32, 32], F32)    # col0 = ue
            UET32 = state.tile([32, 32], F32)    # row0 = ue
            UE10 = state.tile([32, 10], F32)     # ue bcast over partitions
            W0RM = state.tile([10, 512], F16)    # M @ W0r^T (folded)
            WCM = state.tile([10, 10], F16)      # M @ Wc2^T
            scr = state.tile([10, 20], F32)      # accum dst scratch
            rwc16 = state.tile([10, 1], F16)     # f16 copy of rw_col

            for t in (h0, c0, h1rep, c1, M32, Mt32, L32, LT32, zM32, zMr32,
                      RW32, RWT32, WW32, WWT32, Psh, P10b, p10m, UEC32,
                      UET32, UE10, W0RM, WCM, scr):
                nc.vector.memset(t[:], 0.0)
            # ue state must start at DELTA (u=0 <-> ue=delta)
            nc.vector.memset(UEC32[0:10, 0:1], DELTA)


            M = M32[0:10, 0:20]
            Mt = Mt32[0:20, 0:10]
            L = L32[0:10, 0:10]
            LT = LT32[0:10, 0:10]
            zMc = zM32[0:10, 0:1]
            zMr = zMr32[0:1, 0:10]
            rw_row = RW32[0:1, 0:10]
            rw_col = RWT32[0:10, 0:1]
            ww_row = WW32[0:1, 0:10]
            ww_col = WWT32[0:10, 0:1]
            p_row = Psh[0:1, 0:10]
            ue_col = UEC32[0:10, 0:1]

            MASK0 = [0] * 32

            def lstm_cell(G, E_t, c, hout, tag, xb_ap=None):
                """PSUM gates (i,f,g,o sign-folded) -> c/h update; hout is
                [128,1] (cell0) or writes h1rep (cell1 handled by caller)."""
                if xb_ap is not None:
                    z = work.tile([128, 4], F32, tag=f"z{tag}")
                    nc.vector.tensor_tensor(out=z[:], in0=G[:], in1=xb_ap,
                                            op=OP.add)
                    src = z[:]
                else:
                    src = G[:]
                E = work.tile([128, 4], F32, tag=f"E{tag}")
                nc.scalar.activation(E[:], src, AF.Exp)
                D = work.tile([128, 4], F32, tag=f"D{tag}")
                nc.scalar.add(D[:], E[:], 1.0)
                Rr = work.tile([128, 4], F32, tag=f"R{tag}")
                nc.vector.reciprocal(Rr[:], D[:])
                m_ = work.tile([128, 1], F32, tag=f"m{tag}")
                nc.vector.tensor_tensor(out=m_[:], in0=Rr[:, 0:1],
                                        in1=Rr[:, 2:3], op=OP.mult)
                A_ = work.tile([128, 1], F32, tag=f"A{tag}")
                nc.vector.scalar_tensor_tensor(
                    out=A_[:], in0=m_[:], scalar=-2.0, in1=Rr[:, 0:1],
                    op0=OP.mult, op1=OP.add)
                nc.vector.scalar_tensor_tensor(
                    out=c[:], in0=c[:], scalar=Rr[:, 1:2], in1=A_[:],
                    op0=OP.mult, op1=OP.add)
                E2 = work.tile([128, 1], F32, tag=f"E2{tag}")
                nc.scalar.activation(E2[:], c[:], AF.Exp, scale=2.0)
                D2 = work.tile([128, 1], F32, tag=f"D2{tag}")
                nc.scalar.add(D2[:], E2[:], 1.0)
                R2 = work.tile([128, 1], F32, tag=f"R2{tag}")
                nc.vector.reciprocal(R2[:], D2[:])
                m2 = work.tile([128, 1], F32, tag=f"m2{tag}")
                nc.vector.tensor_tensor(out=m2[:], in0=Rr[:, 3:4], in1=R2[:],
                                        op=OP.mult)
                return Rr, m2

            SKIP_W0RM = os.environ.get("SKIP_W0RM", "0") == "1"
            SKIP_MEM = os.environ.get("SKIP_MEM", "0") == "1"
            SKIP_CTRL = os.environ.get("SKIP_CTRL", "0") == "1"

            def step(XB, YT, u):
                # ======== controller cell 0 ========
                if SKIP_CTRL:
                    nc.vector.tensor_copy(YT[:, u:u + 1], XB[0:10, u, 0:1])
                    return
                GG = psG.tile([128, 8], F32, tag="GG")
                G0 = GG[:, 0:4]
                for g in range(4):
                    nc.tensor.matmul(G0[:, g:g + 1],
                                     lhsT=W0HT[:, 128 * g:128 * (g + 1)],
                                     rhs=h0[:], start=True, stop=False)
                    nc.tensor.matmul(G0[:, g:g + 1],
                                     lhsT=W0RM[0:10, 128 * g:128 * (g + 1)],
                                     rhs=rwc16[:], start=False, stop=True)
                Rr0, m20 = lstm_cell(G0, None, c0, h0, "0",
                                     xb_ap=XB[:, 4 * u:4 * u + 4])
                nc.vector.scalar_tensor_tensor(
                    out=h0[:], in0=m20[:], scalar=-2.0, in1=Rr0[:, 3:4],
                    op0=OP.mult, op1=OP.add)

                # ======== controller cell 1 ========
                G1 = GG[:, 4:8]
                for g in range(4):
                    nc.tensor.matmul(G1[:, g:g + 1],
                                     lhsT=W1IT[:, 128 * g:128 * (g + 1)],
                                     rhs=h0[:], start=True, stop=False)
                    nc.tensor.matmul(G1[:, g:g + 1],
                                     lhsT=W1HT[:, 128 * g:128 * (g + 1)],
                                     rhs=h1rep[:, 0:1], start=False, stop=True)
                Rr1, m21 = lstm_cell(G1, None, c1, None, "1")
                th1 = work.tile([128, 1], F32, tag="th1")
                nc.vector.tensor_scalar(out=th1[:], in0=m21[:], scalar1=-2.0,
                                        scalar2=None, op0=OP.mult)
                # h1 replicated x10: (ones*th1)+Ro  per partition
                nc.vector.tensor_scalar(out=h1rep[:], in0=ONES10[:],
                                        scalar1=th1[:], scalar2=Rr1[:, 3:4],
                                        op0=OP.mult, op1=OP.add)

                # ======== interface ========
                XIPS = psA.tile([32, 80], F32, tag="XIPS")
                nc.tensor.matmul(XIPS[0:10, 0:58], lhsT=h1rep[:],
                                 rhs=WXIT[:], start=True, stop=True)
                nc.tensor.matmul(XIPS[0:20, 64:65], lhsT=WRKT[:],
                                 rhs=h1rep[:, 0:1], start=True, stop=True)
                nc.tensor.matmul(XIPS[0:20, 65:66], lhsT=WWKT[:],
                                 rhs=h1rep[:, 0:1], start=True, stop=True)
                EX = work.tile([10, 58], F32, tag="EX")
                nc.scalar.activation(EX[:], XIPS[0:10, 0:58], AF.Exp)
                EK = work.tile([20, 2], F32, tag="EK")
                nc.scalar.activation(EK[:], XIPS[0:20, 64:66], AF.Exp)
                DX = work.tile([10, 58], F32, tag="DX")
                nc.scalar.add(DX[:], EX[:], 1.0)
                RX = work.tile([10, 58], F32, tag="RX")
                nc.vector.reciprocal(RX[:], DX[:])
                tauwv = work.tile([10, 20], F32, tag="tauwv")
                nc.vector.tensor_scalar(out=tauwv[:], in0=RX[:, 0:20],
                                        scalar1=-2.0, scalar2=1.0,
                                        op0=OP.mult, op1=OP.add)
                DK = work.tile([20, 2], F32, tag="DK")
                nc.scalar.add(DK[:], EK[:], 1.0)
                RK = work.tile([20, 2], F32, tag="RK")
                nc.vector.reciprocal(RK[:], DK[:])
                KT = work.tile([20, 2], F32, tag="KT")
                nc.vector.tensor_scalar(out=KT[:], in0=RK[:], scalar1=-2.0,
                                        scalar2=1.0, op0=OP.mult, op1=OP.add)
                # softplus strengths: Ln(1+e^x); col43=rb col44=wb
                BS = work.tile([1, 2], F32, tag="BS")
                nc.scalar.activation(BS[:], DX[0:1, 43:45], AF.Ln)
                # pi softmax pieces (cols 45:48 of EX are exp(pre) already)
                Spi = work.tile([1, 1], F32, tag="Spi")
                nc.vector.tensor_reduce(Spi[:], EX[0:1, 45:48],
                                        axis=AX.X, op=OP.add)
                rSpi = work.tile([1, 1], F32, tag="rSpi")
                nc.vector.reciprocal(rSpi[:], Spi[:])
                qv = work.tile([1, 3], F32, tag="qv")
                nc.vector.tensor_scalar(out=qv[:], in0=EX[0:1, 45:48],
                                        scalar1=rSpi[:], scalar2=None,
                                        op0=OP.mult)
                # early small combines: mga = gw*ga ; c2 = gw - mga
                mga = work.tile([1, 1], F32, tag="mga")
                nc.vector.tensor_tensor(out=mga[:], in0=RX[0:1, 42:43],
                                        in1=RX[0:1, 41:42], op=OP.mult)
                c2 = work.tile([1, 1], F32, tag="c2")
                nc.vector.tensor_tensor(out=c2[:], in0=RX[0:1, 42:43],
                                        in1=mga[:], op=OP.subtract)

                if SKIP_MEM:
                    nc.vector.tensor_copy(YT[:, u:u + 1], RX[0:10, 0:1])
                    return
                # ======== content-write dots on OLD memory (PE) ========
                PSB = psB.tile([32, 64], F32, tag="PSB")
                DOTW = PSB
                nc.tensor.matmul(DOTW[0:1, 0:10], lhsT=KT[:, 1:2],
                                 rhs=Mt, start=True, stop=True)
                nc.tensor.matmul(DOTW[0:1, 10:11], lhsT=KT[:, 1:2],
                                 rhs=KT[:, 1:2], start=True, stop=True)
                nc.tensor.matmul(DOTW[0:1, 11:12], lhsT=KT[:, 0:1],
                                 rhs=KT[:, 0:1], start=True, stop=True)

                # ======== usage + allocation (DVE) ========
                fg = RX[:, 40:41]                 # [10,1] replicated
                # ue' = DELTA + [(ue-DELTA) + ww(1-ue)] * (1 - fg*rw)
                a_ = work.tile([10, 1], F32, tag="a_")
                nc.vector.tensor_scalar(out=a_[:], in0=ue_col, scalar1=-1.0,
                                        scalar2=1.0, op0=OP.mult, op1=OP.add)
                b_ = work.tile([10, 1], F32, tag="b_")
                nc.vector.scalar_tensor_tensor(
                    out=b_[:], in0=a_[:], scalar=ww_col, in1=ue_col,
                    op0=OP.mult, op1=OP.add)      # ue + ww(1-ue)
                bd = work.tile([10, 1], F32, tag="bd")
                nc.vector.tensor_scalar(out=bd[:], in0=b_[:], scalar1=1.0,
                                        scalar2=-DELTA, op0=OP.mult,
                                        op1=OP.add)
                npsi = work.tile([10, 1], F32, tag="npsi")
                nc.vector.scalar_tensor_tensor(
                    out=npsi[:], in0=RWT32[0:10, 0:1], scalar=fg,
                    in1=ONESC[:], op0=OP.mult, op1=OP.subtract)  # fg*rw - 1
                tue = work.tile([10, 1], F32, tag="tue")
                nc.vector.scalar_tensor_tensor(
                    out=tue[:], in0=bd[:], scalar=-1.0, in1=npsi[:],
                    op0=OP.mult, op1=OP.mult)     # bd * psi
                nc.vector.tensor_scalar(out=ue_col, in0=tue[:], scalar1=1.0,
                                        scalar2=DELTA, op0=OP.mult,
                                        op1=OP.add)
                nc.vector.transpose(UET32[:], UEC32[:])
                nc.vector.stream_shuffle(UE10[:], UET32[0:32, 0:10], MASK0)
                eqJ = work.tile([10, 10], F32, tag="eqJ")
                nc.vector.scalar_tensor_tensor(
                    out=eqJ[:], in0=UE10[0:10, :], scalar=ue_col,
                    in1=JLT[:], op0=OP.is_equal, op1=OP.mult)
                cmp = work.tile([10, 10], F32, tag="cmp")
                nc.vector.scalar_tensor_tensor(
                    out=cmp[:], in0=UE10[0:10, :], scalar=ue_col,
                    in1=eqJ[:], op0=OP.is_lt, op1=OP.add)
                tm = work.tile([10, 10], F32, tag="tm")
                nc.vector.tensor_scalar(out=tm[:], in0=UE10[0:10, :],
                                        scalar1=1.0, scalar2=-1.0,
                                        op0=OP.mult, op1=OP.add)  # ue-1
                sel = work.tile([10, 10], F32, tag="sel")
                nc.vector.tensor_tensor(out=sel[:], in0=cmp[:], in1=tm[:],
                                        op=OP.mult)
                nc.vector.tensor_scalar(out=sel[:], in0=sel[:], scalar1=1.0,
                                        scalar2=None, op0=OP.add)
                prod = work.tile([10, 1], F32, tag="prod")
                nc.vector.tensor_reduce(prod[:], sel[:], axis=AX.X,
                                        op=OP.mult)
                s1 = work.tile([10, 1], F32, tag="s1")
                nc.vector.tensor_scalar(out=s1[:], in0=ue_col, scalar1=-1.0,
                                        scalar2=1.0, op0=OP.mult, op1=OP.add)
                AL32 = work.tile([32, 32], F32, tag="AL32")
                nc.vector.tensor_tensor(out=AL32[0:10, 0:1], in0=s1[:],
                                        in1=prod[:], op=OP.mult)
                ALT = work.tile([32, 32], F32, tag="ALT")
                nc.vector.transpose(ALT[:], AL32[:])

                # ======== content write weights (row form) ========
                q2w = work.tile([1, 10], F32, tag="q2w")
                nc.vector.scalar_tensor_tensor(
                    out=q2w[:], in0=zMr, scalar=DOTW[0:1, 10:11],
                    in1=E38[:], op0=OP.mult, op1=OP.max)
                SQW = work.tile([1, 10], F32, tag="SQW")
                nc.scalar.activation(SQW[:], q2w[:], AF.Ln)
                nc.scalar.activation(SQW[:], SQW[:], AF.Exp, scale=0.5)
                dw = work.tile([1, 10], F32, tag="dw")
                nc.vector.tensor_scalar(out=dw[:], in0=SQW[:], scalar1=EPS,
                                        scalar2=None, op0=OP.add)
                rdw = work.tile([1, 10], F32, tag="rdw")
                nc.vector.reciprocal(rdw[:], dw[:])
                simw = work.tile([1, 10], F32, tag="simw")
                nc.vector.tensor_tensor(out=simw[:], in0=DOTW[0:1, 0:10],
                                        in1=rdw[:], op=OP.mult)
                EW = work.tile([1, 10], F32, tag="EW")
                Sw = work.tile([1, 1], F32, tag="Sw")
                nc.scalar.activation(EW[:], simw[:], AF.Exp,
                                     scale=BS[0:1, 1:2], accum_out=Sw[:])
                rSw = work.tile([1, 1], F32, tag="rSw")
                nc.vector.reciprocal(rSw[:], Sw[:])
                te = work.tile([1, 10], F32, tag="te")
                nc.vector.tensor_scalar(out=te[:], in0=EW[:], scalar1=c2[:],
                                        scalar2=rSw[:], op0=OP.mult,
                                        op1=OP.mult)
                sw = work.tile([1, 1], F32, tag="sw")
                nc.vector.scalar_tensor_tensor(
                    out=ww_row, in0=ALT[0:1, 0:10], scalar=mga[:], in1=te[:],
                    op0=OP.mult, op1=OP.add, accum_out=sw[:])
                WW10 = work.tile([32, 10], F32, tag="WW10")
                nc.vector.stream_shuffle(WW10[:], WW32[0:32, 0:10], MASK0)
                nc.vector.transpose(WWT32[:], WW32[:])

                # ======== M update ========
                m1 = work.tile([10, 20], F32, tag="m1")
                nc.vector.scalar_tensor_tensor(
                    out=m1[:], in0=RX[:, 20:40], scalar=ww_col, in1=M,
                    op0=OP.mult, op1=OP.mult)
                M2t = work.tile([10, 20], F32, tag="M2t")
                nc.vector.tensor_tensor(out=M2t[:], in0=M, in1=m1[:],
                                        op=OP.subtract)
                nc.vector.scalar_tensor_tensor(
                    out=M, in0=tauwv[:], scalar=ww_col, in1=M2t[:],
                    op0=OP.mult, op1=OP.add)
                nc.vector.scalar_tensor_tensor(
                    out=scr[:], in0=M, scalar=1.0, in1=M,
                    op0=OP.mult, op1=OP.mult, accum_out=zMc)
                nc.vector.transpose(Mt32[:], M32[:])
                nc.vector.transpose(zMr32[:], zM32[:])
                nc.vector.tensor_copy(Mtr[:], Mt)
                if not SKIP_W0RM:
                    W0RMps = psW.tile([10, 512], F32, tag="W0RMps")
                    nc.tensor.matmul(W0RMps[:], lhsT=Mtr[:], rhs=W0RMT[:],
                                     start=True, stop=True)
                    nc.scalar.copy(W0RM[:], W0RMps[:])

                # ======== L update ========
                lt1 = work.tile([10, 10], F32, tag="lt1")
                nc.vector.scalar_tensor_tensor(
                    out=lt1[:], in0=WW10[0:10, :], scalar=ww_col, in1=L,
                    op0=OP.add, op1=OP.mult)
                lt2 = work.tile([10, 10], F32, tag="lt2")
                nc.vector.tensor_tensor(out=lt2[:], in0=L, in1=lt1[:],
                                        op=OP.subtract)
                nc.vector.scalar_tensor_tensor(
                    out=L, in0=p10m[:], scalar=ww_col, in1=lt2[:],
                    op0=OP.mult, op1=OP.add)
                nc.vector.transpose(LT32[:], L32[:])
                tp = work.tile([1, 10], F32, tag="tp")
                nc.vector.scalar_tensor_tensor(
                    out=tp[:], in0=p_row, scalar=sw[:], in1=ww_row,
                    op0=OP.mult, op1=OP.subtract)
                nc.vector.tensor_tensor(out=p_row, in0=p_row, in1=tp[:],
                                        op=OP.subtract)
                nc.vector.stream_shuffle(P10b[:], Psh[0:32, 0:10], MASK0)
                nc.gpsimd.tensor_tensor(out=p10m[:], in0=P10b[0:10, :],
                                        in1=DIAGM[:], op=OP.mult)

                # ======== read dots (PE, new memory) ========
                nc.tensor.matmul(PSB[0:1, 16:26], lhsT=rw_col, rhs=L,
                                 start=True, stop=True)          # bwd
                nc.tensor.matmul(PSB[0:1, 26:36], lhsT=rw_col, rhs=LT,
                                 start=True, stop=True)          # fwd
                nc.tensor.matmul(PSB[0:1, 36:46], lhsT=KT[:, 0:1], rhs=Mt,
                                 start=True, stop=True)          # dotr

                # ======== content read weights ========
                q2r = work.tile([1, 10], F32, tag="q2r")
                nc.vector.scalar_tensor_tensor(
                    out=q2r[:], in0=zMr, scalar=DOTW[0:1, 11:12],
                    in1=E38[:], op0=OP.mult, op1=OP.max)
                SQR = work.tile([1, 10], F32, tag="SQR")
                nc.scalar.activation(SQR[:], q2r[:], AF.Ln)
                nc.scalar.activation(SQR[:], SQR[:], AF.Exp, scale=0.5)
                dr = work.tile([1, 10], F32, tag="dr")
                nc.vector.tensor_scalar(out=dr[:], in0=SQR[:], scalar1=EPS,
                                        scalar2=None, op0=OP.add)
                rdr = work.tile([1, 10], F32, tag="rdr")
                nc.vector.reciprocal(rdr[:], dr[:])
                simr = work.tile([1, 10], F32, tag="simr")
                nc.vector.tensor_tensor(out=simr[:], in0=PSB[0:1, 36:46],
                                        in1=rdr[:], op=OP.mult)
                ER = work.tile([1, 10], F32, tag="ER")
                Sr = work.tile([1, 1], F32, tag="Sr")
                nc.scalar.activation(ER[:], simr[:], AF.Exp,
                                     scale=BS[0:1, 0:1], accum_out=Sr[:])
                rSr = work.tile([1, 1], F32, tag="rSr")
                nc.vector.reciprocal(rSr[:], Sr[:])
                q1p = work.tile([1, 1], F32, tag="q1p")
                nc.vector.tensor_tensor(out=q1p[:], in0=qv[0:1, 1:2],
                                        in1=rSr[:], op=OP.mult)
                ra = work.tile([1, 10], F32, tag="ra")
                nc.vector.tensor_scalar(out=ra[:], in0=PSB[0:1, 16:26],
                                        scalar1=qv[0:1, 0:1], scalar2=None,
                                        op0=OP.mult)
                rb_ = work.tile([1, 10], F32, tag="rb_")
                nc.vector.scalar_tensor_tensor(
                    out=rb_[:], in0=PSB[0:1, 26:36], scalar=qv[0:1, 2:3],
                    in1=ra[:], op0=OP.mult, op1=OP.add)
                nc.vector.scalar_tensor_tensor(
                    out=rw_row, in0=ER[:], scalar=q1p[:], in1=rb_[:],
                    op0=OP.mult, op1=OP.add)
                nc.vector.transpose(RWT32[:], RW32[:])
                nc.scalar.copy(rwc16[:], RWT32[0:10, 0:1])

                # ======== output y ========
                nc.tensor.matmul(PSB[0:10, 46:56], lhsT=Mt, rhs=WCT2[:],
                                 start=True, stop=True)
                nc.scalar.copy(WCM[:], PSB[0:10, 46:56])
                nc.tensor.matmul(PSB[0:10, 60:61], lhsT=WCM[:], rhs=rwc16[:],
                                 start=True, stop=False)
                nc.tensor.matmul(PSB[0:10, 60:61], lhsT=WCT1[:],
                                 rhs=h1rep[:, 0:1], start=False, stop=True)
                nc.scalar.copy(YT[:, u:u + 1], PSB[0:10, 60:61])



            # ================= main loop =================
            import concourse.mybir as _mb
            _stag = os.environ.get("STAG", "1") == "1"
            _hints = () if os.environ.get("NO_HINTS", "0") == "1" else tuple(_mb.ALL_ENGINES)
            with tc.For_i(0, T // U, 1, hint_engines=_hints,
                          staggered_reset=_stag) as iv:
                XB = io.tile([128, 4 * U], F32, tag="XB")
                nc.sync.dma_start(XB[:], d_xb3[:, ds(iv, 1), :])
                YT = io.tile([10, U], F32, tag="YT")
                for u in range(U):
                    step(XB, YT, u)
                nc.gpsimd.dma_start(d_yt[:, ds(iv, 1), :], YT[:])

    nc.compile()
    return nc, names


def prep_inputs(inputs, T=SEQ_LEN, U=8):
    f = lambda k: np.asarray(inputs[k], dtype=np.float32)
    x = f("x")[:T]
    W_ih0, W_hh0 = f("W_ih0"), f("W_hh0")
    W_ih1, W_hh1 = f("W_ih1"), f("W_hh1")
    W_xi = f("W_xi")
    W_out, W_fc = f("W_out"), f("W_fc")
    for k in ("b_ih0", "b_hh0", "b_ih1", "b_hh1", "b_xi", "b_out", "b_fc"):
        assert np.abs(f(k)).max() == 0.0, f"kernel assumes {k} == 0"

    gs = np.repeat(np.array([-1.0, -1.0, 2.0, -1.0], np.float32), H)  # (512,)

    xb = (x @ W_ih0[:, :X_DIM].T) * gs[None, :]
    # [128, T//U, U*4]: chunk-contiguous per partition, (u, g) order
    xb3 = np.ascontiguousarray(
        xb.reshape(T // U, U, 4, H).transpose(3, 0, 1, 2)
        .reshape(H, T // U, 4 * U))

    w0ht = np.ascontiguousarray((W_hh0 * gs[:, None]).T)              # 128x512
    w0rmt = np.ascontiguousarray((W_ih0[:, X_DIM:] * gs[:, None]).T)  # 20x512
    w1it = np.ascontiguousarray((W_ih1 * gs[:, None]).T)
    w1ht = np.ascontiguousarray((W_hh1 * gs[:, None]).T)

    # interface: cols [wv(20) tanh | er(20) sig | fg ga gw sig | rb wb sp |
    #                  pi(3) raw]
    idx = np.concatenate([
        np.arange(62, 82),        # wv
        np.arange(42, 62),        # er
        [82, 83, 84],             # fg ga gw
        [20, 41],                 # rb wb
        [85, 86, 87],             # pi
        [82] * 10,                # fg x10 (free-dim broadcast)
    ]).astype(np.int64)
    sc = np.concatenate([
        2.0 * np.ones(20), -1.0 * np.ones(20), -1.0 * np.ones(3),
        np.ones(2), np.ones(3), -1.0 * np.ones(10),
    ]).astype(np.float32)
    wxit = np.ascontiguousarray((W_xi[idx] * sc[:, None]).T)          # 128x48
    wrkt = np.ascontiguousarray((W_xi[0:20] * 2.0).T)                 # 128x20
    wwkt = np.ascontiguousarray((W_xi[21:41] * 2.0).T)                # 128x20

    Wcomb = W_fc @ W_out                                              # 10x148
    wct1 = np.ascontiguousarray(Wcomb[:, :H].T)                       # 128x10
    wct2 = np.ascontiguousarray(Wcomb[:, H:].T)                       # 20x10

    jlt = np.tril(np.ones((10, 10), np.float32), -1)
    diagm = (1.0 - np.eye(10)).astype(np.float32)

    f16 = np.float16
    return {
        "xb3": xb3, "w0ht": w0ht.astype(f16), "w0rmt": w0rmt,
        "w1it": w1it.astype(f16), "w1ht": w1ht.astype(f16),
        "wxit": wxit.astype(f16), "wrkt": wrkt.astype(f16),
        "wwkt": wwkt.astype(f16), "wct1": wct1.astype(f16),
        "wct2": wct2, "jlt": jlt, "diagm": diagm,
    }


_BUILD_CACHE = {}


def kernel(**inputs):
    T = np.asarray(inputs["x"]).shape[0]
    U = 8
    key = (T, U)
    if key not in _BUILD_CACHE:
        _BUILD_CACHE[key] = build(T=T, U=U)
    nc, names = _BUILD_CACHE[key]
    logical = prep_inputs(inputs, T=T, U=U)
    in_map = {names[k]: v for k, v in logical.items()}

    from concourse.bass_utils import run_bass_kernel_spmd
    res = run_bass_kernel_spmd(nc, [in_map], core_ids=[0])
    yt = res.results[0][names["yt"]]          # (10, T//U, U)
    y = np.asarray(yt).reshape(10, T)
    return np.ascontiguousarray(y.T)[None].astype(np.float32)


# revision 4
# speedup vs baseline: 1.8491x; 1.8491x over previous
"""DNC sequential kernel v2 for Trainium2 — row-major redesign.

Single-core Bass/Tile implementation of the 8192-step DNC recurrence.
Design vs the v1 baseline:
- Replicated-output PE matmuls give the interface activations pre-broadcast
  over the 10 memory cells; no gpsimd partition_broadcast anywhere.
- stream_shuffle (DVE) does the remaining partition broadcasts.
- All dot products via scalar_tensor_tensor + accum_out (1 DVE op) or tiny
  PE matmuls in row form.
- Sort-free allocation: pairwise-compare matrix + product-reduce (DVE
  tensor_reduce with mult), stable-argsort tie semantics via is_equal*JLT.
- Read vector rv is never materialized: gates0 use W0rM = M @ W_ih0[:,10:]^T
  (f32r matmul, precomputed per step once M is updated) applied to rw, and
  the output projection uses WcM = M @ Wc2^T applied to rw.
- ACT table set 6 (exp/ln) preloaded once; sigmoid/tanh via exp with signs
  folded into host weights; softplus = Ln(1+e^x) exact; sqrt via Ln+Exp(0.5).
- All biases in this problem are zero (asserted on host) and dropped.
- Controller-side matmuls (gates, interface, output proj) use fp16
  stationaries + fp16 hidden states: fp32 [.,128] weight loads cost ~800ns
  on the PE, fp16 loads ~4x less; all DNC memory-state math stays fp32.
- Input xb3 is chunk-contiguous [128, T/U, 4U] so each chunk DMA is 128
  large descriptors instead of 512 16-byte ones; output DMA issues from
  the gpsimd queue so input prefetch on SP runs a chunk ahead.
"""

import sys
import os
import numpy as np

if "/opt/trn_rl_repo" not in sys.path:
    sys.path.insert(0, "/opt/trn_rl_repo")

N, CELL, R, H, X_DIM, OUT_DIM = 10, 20, 1, 128, 10, 10
EPS, DELTA = 1e-6, 1e-6
SEQ_LEN = 8192


def build(T=SEQ_LEN, U=8):
    import concourse.bass as bass
    import concourse.bacc as bacc
    import concourse.mybir as mybir
    from concourse import tile

    F32 = mybir.dt.float32
    F32R = mybir.dt.float32r
    F16 = mybir.dt.float16
    AF = mybir.ActivationFunctionType
    OP = mybir.AluOpType
    AX = mybir.AxisListType
    ds = bass.ds

    assert T % U == 0

    nc = bacc.Bacc(None, target_bir_lowering=False, debug=False)
    names = {}

    with tile.TileContext(nc) as tc:
        with tc.tile_pool(name="dram", bufs=1, space="DRAM") as dram, \
             tc.tile_pool(name="consts", bufs=1) as consts, \
             tc.tile_pool(name="state", bufs=1) as state, \
             tc.tile_pool(name="work", bufs=2) as work, \
             tc.tile_pool(name="io", bufs=2) as io, \
             tc.tile_pool(name="psG", bufs=2, space="PSUM") as psG, \
             tc.tile_pool(name="psA", bufs=2, space="PSUM") as psA, \
             tc.tile_pool(name="psB", bufs=2, space="PSUM") as psB, \
             tc.tile_pool(name="psW", bufs=1, space="PSUM") as psW:

            # ---------------- DRAM I/O ----------------
            d_xb3 = dram.tile([128, T // U, 4 * U], F32, kind="ExternalInput")
            d_w0ht = dram.tile([128, 512], F16, kind="ExternalInput")
            d_w0rmt = dram.tile([20, 512], F32R, kind="ExternalInput")
            d_w1it = dram.tile([128, 512], F16, kind="ExternalInput")
            d_w1ht = dram.tile([128, 512], F16, kind="ExternalInput")
            d_wxit = dram.tile([128, 58], F16, kind="ExternalInput")
            d_wrkt = dram.tile([128, 20], F16, kind="ExternalInput")
            d_wwkt = dram.tile([128, 20], F16, kind="ExternalInput")
            d_wct1 = dram.tile([128, 10], F16, kind="ExternalInput")
            d_wct2 = dram.tile([20, 10], F32, kind="ExternalInput")
            d_jlt = dram.tile([10, 10], F32, kind="ExternalInput")
            d_diagm = dram.tile([10, 10], F32, kind="ExternalInput")
            d_yt = dram.tile([10, T // U, U], F32, kind="ExternalOutput")
            for k, v in [("xb3", d_xb3), ("w0ht", d_w0ht), ("w0rmt", d_w0rmt),
                         ("w1it", d_w1it), ("w1ht", d_w1ht), ("wxit", d_wxit),
                         ("wrkt", d_wrkt), ("wwkt", d_wwkt), ("wct1", d_wct1),
                         ("wct2", d_wct2),
                         ("jlt", d_jlt), ("diagm", d_diagm),
                         ("yt", d_yt)]:
                names[k] = v.tensor.name

            # preload ACT set 6 (exp/ln) once; all activations stay in-set
            nc.scalar.add_instruction(mybir.InstLoadActFuncSet(
                name=nc.get_next_instruction_name(), act_func_set_id=6,
                ins=[], outs=[]))

            # ---------------- const SBUF ----------------
            W0HT = consts.tile([128, 512], F16)
            W0RMT = consts.tile([20, 512], F32R)
            W1IT = consts.tile([128, 512], F16)
            W1HT = consts.tile([128, 512], F16)
            WXIT = consts.tile([128, 58], F16)
            WRKT = consts.tile([128, 20], F16)
            WWKT = consts.tile([128, 20], F16)
            WCT1 = consts.tile([128, 10], F16)
            WCT2 = consts.tile([20, 10], F32)
            JLT = consts.tile([10, 10], F32)
            DIAGM = consts.tile([10, 10], F32)
            E38 = consts.tile([1, 10], F32)
            ONESC = consts.tile([10, 1], F32)
            ONES10 = consts.tile([128, 10], F32)

            for dst, src in [(W0HT, d_w0ht), (W0RMT, d_w0rmt), (W1IT, d_w1it),
                             (W1HT, d_w1ht), (WXIT, d_wxit), (WRKT, d_wrkt),
                             (WWKT, d_wwkt), (WCT1, d_wct1), (WCT2, d_wct2),
                             (JLT, d_jlt), (DIAGM, d_diagm)]:
                nc.sync.dma_start(dst[:], src[:])
            nc.vector.memset(E38[:], 1e-36)
            nc.vector.memset(ONESC[:], 1.0)
            nc.vector.memset(ONES10[:], 1.0)

            # ---------------- state SBUF ----------------
            h0 = state.tile([128, 1], F16)
            c0 = state.tile([128, 1], F32)
            h1rep = state.tile([128, 10], F16)   # h1 replicated; col 0 = h1
            c1 = state.tile([128, 1], F32)
            M32 = state.tile([32, 32], F32)      # M  = [0:10, 0:20]
            Mt32 = state.tile([32, 32], F32)     # Mt = [0:20, 0:10]
            L32 = state.tile([32, 32], F32)      # L  = [0:10, 0:10]
            LT32 = state.tile([32, 32], F32)
            Mtr = state.tile([20, 10], F32R)     # f32r copy of Mt
            zM32 = state.tile([32, 32], F32)     # col 0 = zM (row norms^2)
            zMr32 = state.tile([32, 32], F32)    # row 0 = zM as a row
            RW32 = state.tile([32, 32], F32)     # row 0 = rw_row
            RWT32 = state.tile([32, 32], F32)    # col 0 = rw_col
            WW32 = state.tile([32, 32], F32)     # row 0 = ww_row
            WWT32 = state.tile([32, 32], F32)    # col 0 = ww_col
            Psh = state.tile([32, 32], F32)      # row 0 = p_row
            P10b = state.tile([32, 10], F32)     # p broadcast
            p10m = state.tile([10, 10], F32)     # p bcast, diag-masked
            UEC32 = state.tile([32, 32], F32)    # col0 = ue
            UET32 = state.tile([32, 32], F32)    # row0 = ue
            UE10 = state.tile([32, 10], F32)     # ue bcast over partitions
            W0RM = state.tile([10, 512], F16)    # M @ W0r^T (folded)
            WCM = state.tile([10, 10], F16)      # M @ Wc2^T
            scr = state.tile([10, 20], F32)      # accum dst scratch
            rwc16 = state.tile([10, 1], F16)     # f16 copy of rw_col

            for t in (h0, c0, h1rep, c1, M32, Mt32, L32, LT32, zM32, zMr32,
                      RW32, RWT32, WW32, WWT32, Psh, P10b, p10m, UEC32,
                      UET32, UE10, W0RM, WCM, scr):
                nc.vector.memset(t[:], 0.0)
            # ue state must start at DELTA (u=0 <-> ue=delta)
            nc.vector.memset(UEC32[0:10, 0:1], DELTA)


            M = M32[0:10, 0:20]
            Mt = Mt32[0:20, 0:10]
            L = L32[0:10, 0:10]
            LT = LT32[0:10, 0:10]
            zMc = zM32[0:10, 0:1]
            zMr = zMr32[0:1, 0:10]
            rw_row = RW32[0:1, 0:10]
            rw_col = RWT32[0:10, 0:1]
            ww_row = WW32[0:1, 0:10]
            ww_col = WWT32[0:10, 0:1]
            p_row = Psh[0:1, 0:10]
            ue_col = UEC32[0:10, 0:1]

            MASK0 = [0] * 32

            def lstm_cell(G, E_t, c, hout, tag, xb_ap=None):
                """PSUM gates (i,f,g,o sign-folded) -> c/h update; hout is
                [128,1] (cell0) or writes h1rep (cell1 handled by caller)."""
                if xb_ap is not None:
                    z = work.tile([128, 4], F32, tag=f"z{tag}")
                    nc.vector.tensor_tensor(out=z[:], in0=G[:], in1=xb_ap,
                                            op=OP.add)
                    src = z[:]
                else:
                    src = G[:]
                E = work.tile([128, 4], F32, tag=f"E{tag}")
                nc.scalar.activation(E[:], src, AF.Exp)
                D = work.tile([128, 4], F32, tag=f"D{tag}")
                nc.scalar.add(D[:], E[:], 1.0)
                Rr = work.tile([128, 4], F32, tag=f"R{tag}")
                nc.vector.reciprocal(Rr[:], D[:])
                m_ = work.tile([128, 1], F32, tag=f"m{tag}")
                nc.vector.tensor_tensor(out=m_[:], in0=Rr[:, 0:1],
                                        in1=Rr[:, 2:3], op=OP.mult)
                A_ = work.tile([128, 1], F32, tag=f"A{tag}")
                nc.vector.scalar_tensor_tensor(
                    out=A_[:], in0=m_[:], scalar=-2.0, in1=Rr[:, 0:1],
                    op0=OP.mult, op1=OP.add)
                nc.vector.scalar_tensor_tensor(
                    out=c[:], in0=c[:], scalar=Rr[:, 1:2], in1=A_[:],
                    op0=OP.mult, op1=OP.add)
                E2 = work.tile([128, 1], F32, tag=f"E2{tag}")
                nc.scalar.activation(E2[:], c[:], AF.Exp, scale=2.0)
                D2 = work.tile([128, 1], F32, tag=f"D2{tag}")
                nc.scalar.add(D2[:], E2[:], 1.0)
                R2 = work.tile([128, 1], F32, tag=f"R2{tag}")
                nc.vector.reciprocal(R2[:], D2[:])
                m2 = work.tile([128, 1], F32, tag=f"m2{tag}")
                nc.vector.tensor_tensor(out=m2[:], in0=Rr[:, 3:4], in1=R2[:],
                                        op=OP.mult)
                return Rr, m2

            SKIP_W0RM = os.environ.get("SKIP_W0RM", "0") == "1"
            SKIP_MEM = os.environ.get("SKIP_MEM", "0") == "1"
            SKIP_CTRL = os.environ.get("SKIP_CTRL", "0") == "1"

            def step(XB, YT, u):
                # ======== controller cell 0 ========
                if SKIP_CTRL:
                    nc.vector.tensor_copy(YT[:, u:u + 1], XB[0:10, u, 0:1])
                    return
                GG = psG.tile([128, 8], F32, tag="GG")
                G0 = GG[:, 0:4]
                for g in range(4):
                    nc.tensor.matmul(G0[:, g:g + 1],
                                     lhsT=W0HT[:, 128 * g:128 * (g + 1)],
                                     rhs=h0[:], start=True, stop=False)
                    nc.tensor.matmul(G0[:, g:g + 1],
                                     lhsT=W0RM[0:10, 128 * g:128 * (g + 1)],
                                     rhs=rwc16[:], start=False, stop=True)
                Rr0, m20 = lstm_cell(G0, None, c0, h0, "0",
                                     xb_ap=XB[:, 4 * u:4 * u + 4])
                nc.vector.scalar_tensor_tensor(
                    out=h0[:], in0=m20[:], scalar=-2.0, in1=Rr0[:, 3:4],
                    op0=OP.mult, op1=OP.add)

                # ======== controller cell 1 ========
                G1 = GG[:, 4:8]
                for g in range(4):
                    nc.tensor.matmul(G1[:, g:g + 1],
                                     lhsT=W1IT[:, 128 * g:128 * (g + 1)],
                                     rhs=h0[:], start=True, stop=False)
                    nc.tensor.matmul(G1[:, g:g + 1],
                                     lhsT=W1HT[:, 128 * g:128 * (g + 1)],
                                     rhs=h1rep[:, 0:1], start=False, stop=True)
                Rr1, m21 = lstm_cell(G1, None, c1, None, "1")
                th1 = work.tile([128, 1], F32, tag="th1")
                nc.vector.tensor_scalar(out=th1[:], in0=m21[:], scalar1=-2.0,
                                        scalar2=None, op0=OP.mult)
                # h1 replicated x10: (ones*th1)+Ro  per partition
                nc.vector.tensor_scalar(out=h1rep[:], in0=ONES10[:],
                                        scalar1=th1[:], scalar2=Rr1[:, 3:4],
                                        op0=OP.mult, op1=OP.add)

                # ======== interface ========
                XIPS = psA.tile([32, 80], F32, tag="XIPS")
                nc.tensor.matmul(XIPS[0:10, 0:58], lhsT=h1rep[:],
                                 rhs=WXIT[:], start=True, stop=True)
                nc.tensor.matmul(XIPS[0:20, 64:65], lhsT=WRKT[:],
                                 rhs=h1rep[:, 0:1], start=True, stop=True)
                nc.tensor.matmul(XIPS[0:20, 65:66], lhsT=WWKT[:],
                                 rhs=h1rep[:, 0:1], start=True, stop=True)
                EX = work.tile([10, 58], F32, tag="EX")
                nc.scalar.activation(EX[:], XIPS[0:10, 0:58], AF.Exp)
                EK = work.tile([20, 2], F32, tag="EK")
                nc.scalar.activation(EK[:], XIPS[0:20, 64:66], AF.Exp)
                DX = work.tile([10, 58], F32, tag="DX")
                nc.scalar.add(DX[:], EX[:], 1.0)
                RX = work.tile([10, 58], F32, tag="RX")
                nc.vector.reciprocal(RX[:], DX[:])
                tauwv = work.tile([10, 20], F32, tag="tauwv")
                nc.vector.tensor_scalar(out=tauwv[:], in0=RX[:, 0:20],
                                        scalar1=-2.0, scalar2=1.0,
                                        op0=OP.mult, op1=OP.add)
                DK = work.tile([20, 2], F32, tag="DK")
                nc.scalar.add(DK[:], EK[:], 1.0)
                RK = work.tile([20, 2], F32, tag="RK")
                nc.vector.reciprocal(RK[:], DK[:])
                KT = work.tile([20, 2], F32, tag="KT")
                nc.vector.tensor_scalar(out=KT[:], in0=RK[:], scalar1=-2.0,
                                        scalar2=1.0, op0=OP.mult, op1=OP.add)
                # softplus strengths: Ln(1+e^x); col43=rb col44=wb
                BS = work.tile([1, 2], F32, tag="BS")
                nc.scalar.activation(BS[:], DX[0:1, 43:45], AF.Ln)
                # pi softmax pieces (cols 45:48 of EX are exp(pre) already)
                Spi = work.tile([1, 1], F32, tag="Spi")
                nc.vector.tensor_reduce(Spi[:], EX[0:1, 45:48],
                                        axis=AX.X, op=OP.add)
                rSpi = work.tile([1, 1], F32, tag="rSpi")
                nc.vector.reciprocal(rSpi[:], Spi[:])
                qv = work.tile([1, 3], F32, tag="qv")
                nc.vector.tensor_scalar(out=qv[:], in0=EX[0:1, 45:48],
                                        scalar1=rSpi[:], scalar2=None,
                                        op0=OP.mult)
                # early small combines: mga = gw*ga ; c2 = gw - mga
                mga = work.tile([1, 1], F32, tag="mga")
                nc.vector.tensor_tensor(out=mga[:], in0=RX[0:1, 42:43],
                                        in1=RX[0:1, 41:42], op=OP.mult)
                c2 = work.tile([1, 1], F32, tag="c2")
                nc.vector.tensor_tensor(out=c2[:], in0=RX[0:1, 42:43],
                                        in1=mga[:], op=OP.subtract)

                if SKIP_MEM:
                    nc.vector.tensor_copy(YT[:, u:u + 1], RX[0:10, 0:1])
                    return
                # ======== content-write dots on OLD memory (PE) ========
                PSB = psB.tile([32, 64], F32, tag="PSB")
                DOTW = PSB
                nc.tensor.matmul(DOTW[0:1, 0:10], lhsT=KT[:, 1:2],
                                 rhs=Mt, start=True, stop=True)
                nc.tensor.matmul(DOTW[0:1, 10:11], lhsT=KT[:, 1:2],
                                 rhs=KT[:, 1:2], start=True, stop=True)
                nc.tensor.matmul(DOTW[0:1, 11:12], lhsT=KT[:, 0:1],
                                 rhs=KT[:, 0:1], start=True, stop=True)

                # ======== usage + allocation (DVE) ========
                fg = RX[:, 40:41]                 # [10,1] replicated
                # ue' = DELTA + [(ue-DELTA) + ww(1-ue)] * (1 - fg*rw)
                a_ = work.tile([10, 1], F32, tag="a_")
                nc.vector.tensor_scalar(out=a_[:], in0=ue_col, scalar1=-1.0,
                                        scalar2=1.0, op0=OP.mult, op1=OP.add)
                b_ = work.tile([10, 1], F32, tag="b_")
                nc.vector.scalar_tensor_tensor(
                    out=b_[:], in0=a_[:], scalar=ww_col, in1=ue_col,
                    op0=OP.mult, op1=OP.add)      # ue + ww(1-ue)
                bd = work.tile([10, 1], F32, tag="bd")
                nc.vector.tensor_scalar(out=bd[:], in0=b_[:], scalar1=1.0,
                                        scalar2=-DELTA, op0=OP.mult,
                                        op1=OP.add)
                npsi = work.tile([10, 1], F32, tag="npsi")
                nc.vector.scalar_tensor_tensor(
                    out=npsi[:], in0=RWT32[0:10, 0:1], scalar=fg,
                    in1=ONESC[:], op0=OP.mult, op1=OP.subtract)  # fg*rw - 1
                tue = work.tile([10, 1], F32, tag="tue")
                nc.vector.scalar_tensor_tensor(
                    out=tue[:], in0=bd[:], scalar=-1.0, in1=npsi[:],
                    op0=OP.mult, op1=OP.mult)     # bd * psi
                nc.vector.tensor_scalar(out=ue_col, in0=tue[:], scalar1=1.0,
                                        scalar2=DELTA, op0=OP.mult,
                                        op1=OP.add)
                nc.vector.transpose(UET32[:], UEC32[:])
                nc.vector.stream_shuffle(UE10[:], UET32[0:32, 0:10], MASK0)
                eqJ = work.tile([10, 10], F32, tag="eqJ")
                nc.vector.scalar_tensor_tensor(
                    out=eqJ[:], in0=UE10[0:10, :], scalar=ue_col,
                    in1=JLT[:], op0=OP.is_equal, op1=OP.mult)
                cmp = work.tile([10, 10], F32, tag="cmp")
                nc.vector.scalar_tensor_tensor(
                    out=cmp[:], in0=UE10[0:10, :], scalar=ue_col,
                    in1=eqJ[:], op0=OP.is_lt, op1=OP.add)
                tm = work.tile([10, 10], F32, tag="tm")
                nc.vector.tensor_scalar(out=tm[:], in0=UE10[0:10, :],
                                        scalar1=1.0, scalar2=-1.0,
                                        op0=OP.mult, op1=OP.add)  # ue-1
                sel = work.tile([10, 10], F32, tag="sel")
                nc.vector.tensor_tensor(out=sel[:], in0=cmp[:], in1=tm[:],
                                        op=OP.mult)
                nc.vector.tensor_scalar(out=sel[:], in0=sel[:], scalar1=1.0,
                                        scalar2=None, op0=OP.add)
                prod = work.tile([10, 1], F32, tag="prod")
                nc.vector.tensor_reduce(prod[:], sel[:], axis=AX.X,
                                        op=OP.mult)
                s1 = work.tile([10, 1], F32, tag="s1")
                nc.vector.tensor_scalar(out=s1[:], in0=ue_col, scalar1=-1.0,
                                        scalar2=1.0, op0=OP.mult, op1=OP.add)
                AL32 = work.tile([32, 32], F32, tag="AL32")
                nc.vector.tensor_tensor(out=AL32[0:10, 0:1], in0=s1[:],
                                        in1=prod[:], op=OP.mult)
                ALT = work.tile([32, 32], F32, tag="ALT")
                nc.vector.transpose(ALT[:], AL32[:])

                # ======== content write weights (row form) ========
                q2w = work.tile([1, 10], F32, tag="q2w")
                nc.vector.scalar_tensor_tensor(
                    out=q2w[:], in0=zMr, scalar=DOTW[0:1, 10:11],
                    in1=E38[:], op0=OP.mult, op1=OP.max)
                SQW = work.tile([1, 10], F32, tag="SQW")
                nc.scalar.activation(SQW[:], q2w[:], AF.Ln)
                nc.scalar.activation(SQW[:], SQW[:], AF.Exp, scale=0.5)
                dw = work.tile([1, 10], F32, tag="dw")
                nc.vector.tensor_scalar(out=dw[:], in0=SQW[:], scalar1=EPS,
                                        scalar2=None, op0=OP.add)
                rdw = work.tile([1, 10], F32, tag="rdw")
                nc.vector.reciprocal(rdw[:], dw[:])
                simw = work.tile([1, 10], F32, tag="simw")
                nc.vector.tensor_tensor(out=simw[:], in0=DOTW[0:1, 0:10],
                                        in1=rdw[:], op=OP.mult)
                EW = work.tile([1, 10], F32, tag="EW")
                Sw = work.tile([1, 1], F32, tag="Sw")
                nc.scalar.activation(EW[:], simw[:], AF.Exp,
                                     scale=BS[0:1, 1:2], accum_out=Sw[:])
                rSw = work.tile([1, 1], F32, tag="rSw")
                nc.vector.reciprocal(rSw[:], Sw[:])
                te = work.tile([1, 10], F32, tag="te")
                nc.vector.tensor_scalar(out=te[:], in0=EW[:], scalar1=c2[:],
                                        scalar2=rSw[:], op0=OP.mult,
                                        op1=OP.mult)
                sw = work.tile([1, 1], F32, tag="sw")
                nc.vector.scalar_tensor_tensor(
                    out=ww_row, in0=ALT[0:1, 0:10], scalar=mga[:], in1=te[:],
                    op0=OP.mult, op1=OP.add, accum_out=sw[:])
                WW10 = work.tile([32, 10], F32, tag="WW10")
                nc.vector.stream_shuffle(WW10[:], WW32[0:32, 0:10], MASK0)
                nc.vector.transpose(WWT32[:], WW32[:])

                # ======== M update ========
                m1 = work.tile([10, 20], F32, tag="m1")
                nc.vector.scalar_tensor_tensor(
                    out=m1[:], in0=RX[:, 20:40], scalar=ww_col, in1=M,
                    op0=OP.mult, op1=OP.mult)
                M2t = work.tile([10, 20], F32, tag="M2t")
                nc.vector.tensor_tensor(out=M2t[:], in0=M, in1=m1[:],
                                        op=OP.subtract)
                nc.vector.scalar_tensor_tensor(
                    out=M, in0=tauwv[:], scalar=ww_col, in1=M2t[:],
                    op0=OP.mult, op1=OP.add)
                nc.vector.scalar_tensor_tensor(
                    out=scr[:], in0=M, scalar=1.0, in1=M,
                    op0=OP.mult, op1=OP.mult, accum_out=zMc)
                nc.vector.transpose(Mt32[:], M32[:])
                nc.vector.transpose(zMr32[:], zM32[:])
                nc.vector.tensor_copy(Mtr[:], Mt)
                if not SKIP_W0RM:
                    W0RMps = psW.tile([10, 512], F32, tag="W0RMps")
                    nc.tensor.matmul(W0RMps[:], lhsT=Mtr[:], rhs=W0RMT[:],
                                     start=True, stop=True)
                    nc.scalar.copy(W0RM[:], W0RMps[:])

                # ======== L update ========
                lt1 = work.tile([10, 10], F32, tag="lt1")
                nc.vector.scalar_tensor_tensor(
                    out=lt1[:], in0=WW10[0:10, :], scalar=ww_col, in1=L,
                    op0=OP.add, op1=OP.mult)
                lt2 = work.tile([10, 10], F32, tag="lt2")
                nc.vector.tensor_tensor(out=lt2[:], in0=L, in1=lt1[:],
                                        op=OP.subtract)
                nc.vector.scalar_tensor_tensor(
                    out=L, in0=p10m[:], scalar=ww_col, in1=lt2[:],
                    op0=OP.mult, op1=OP.add)
                nc.vector.transpose(LT32[:], L32[:])
                tp = work.tile([1, 10], F32, tag="tp")
                nc.vector.scalar_tensor_tensor(
                    out=tp[:], in0=p_row, scalar=sw[:], in1=ww_row,
                    op0=OP.mult, op1=OP.subtract)
                nc.vector.tensor_tensor(out=p_row, in0=p_row, in1=tp[:],
                                        op=OP.subtract)
                nc.vector.stream_shuffle(P10b[:], Psh[0:32, 0:10], MASK0)
                nc.gpsimd.tensor_tensor(out=p10m[:], in0=P10b[0:10, :],
                                        in1=DIAGM[:], op=OP.mult)

                # ======== read dots (PE, new memory) ========
                nc.tensor.matmul(PSB[0:1, 16:26], lhsT=rw_col, rhs=L,
                                 start=True, stop=True)          # bwd
                nc.tensor.matmul(PSB[0:1, 26:36], lhsT=rw_col, rhs=LT,
                                 start=True, stop=True)          # fwd
                nc.tensor.matmul(PSB[0:1, 36:46], lhsT=KT[:, 0:1], rhs=Mt,
                                 start=True, stop=True)          # dotr

                # ======== content read weights ========
                q2r = work.tile([1, 10], F32, tag="q2r")
                nc.vector.scalar_tensor_tensor(
                    out=q2r[:], in0=zMr, scalar=DOTW[0:1, 11:12],
                    in1=E38[:], op0=OP.mult, op1=OP.max)
                SQR = work.tile([1, 10], F32, tag="SQR")
                nc.scalar.activation(SQR[:], q2r[:], AF.Ln)
                nc.scalar.activation(SQR[:], SQR[:], AF.Exp, scale=0.5)
                dr = work.tile([1, 10], F32, tag="dr")
                nc.vector.tensor_scalar(out=dr[:], in0=SQR[:], scalar1=EPS,
                                        scalar2=None, op0=OP.add)
                rdr = work.tile([1, 10], F32, tag="rdr")
                nc.vector.reciprocal(rdr[:], dr[:])
                simr = work.tile([1, 10], F32, tag="simr")
                nc.vector.tensor_tensor(out=simr[:], in0=PSB[0:1, 36:46],
                                        in1=rdr[:], op=OP.mult)
                ER = work.tile([1, 10], F32, tag="ER")
                Sr = work.tile([1, 1], F32, tag="Sr")
                nc.scalar.activation(ER[:], simr[:], AF.Exp,
                                     scale=BS[0:1, 0:1], accum_out=Sr[:])
                rSr = work.tile([1, 1], F32, tag="rSr")
                nc.vector.reciprocal(rSr[:], Sr[:])
                q1p = work.tile([1, 1], F32, tag="q1p")
                nc.vector.tensor_tensor(out=q1p[:], in0=qv[0:1, 1:2],
                                        in1=rSr[:], op=OP.mult)
                ra = work.tile([1, 10], F32, tag="ra")
                nc.vector.tensor_scalar(out=ra[:], in0=PSB[0:1, 16:26],
                                        scalar1=qv[0:1, 0:1], scalar2=None,
                                        op0=OP.mult)
                rb_ = work.tile([1, 10], F32, tag="rb_")
                nc.vector.scalar_tensor_tensor(
                    out=rb_[:], in0=PSB[0:1, 26:36], scalar=qv[0:1, 2:3],
                    in1=ra[:], op0=OP.mult, op1=OP.add)
                nc.vector.scalar_tensor_tensor(
                    out=rw_row, in0=ER[:], scalar=q1p[:], in1=rb_[:],
                    op0=OP.mult, op1=OP.add)
                nc.vector.transpose(RWT32[:], RW32[:])
                nc.scalar.copy(rwc16[:], RWT32[0:10, 0:1])

                # ======== output y ========
                nc.tensor.matmul(PSB[0:10, 46:56], lhsT=Mt, rhs=WCT2[:],
                                 start=True, stop=True)
                nc.scalar.copy(WCM[:], PSB[0:10, 46:56])
                nc.tensor.matmul(PSB[0:10, 60:61], lhsT=WCM[:], rhs=rwc16[:],
                                 start=True, stop=False)
                nc.tensor.matmul(PSB[0:10, 60:61], lhsT=WCT1[:],
                                 rhs=h1rep[:, 0:1], start=False, stop=True)
                nc.scalar.copy(YT[:, u:u + 1], PSB[0:10, 60:61])



            # ================= main loop =================
            import concourse.mybir as _mb
            _stag = os.environ.get("STAG", "1") == "1"
            _hints = () if os.environ.get("NO_HINTS", "0") == "1" else tuple(_mb.ALL_ENGINES)
            with tc.For_i(0, T // U, 1, hint_engines=_hints,
                          staggered_reset=_stag) as iv:
                XB = io.tile([128, 4 * U], F32, tag="XB")
                nc.sync.dma_start(XB[:], d_xb3[:, ds(iv, 1), :])
                YT = io.tile([10, U], F32, tag="YT")
                for u in range(U):
                    step(XB, YT, u)
                nc.gpsimd.dma_start(d_yt[:, ds(iv, 1), :], YT[:])

    nc.compile()
    return nc, names


def prep_inputs(inputs, T=SEQ_LEN, U=8):
    f = lambda k: np.asarray(inputs[k], dtype=np.float32)
    x = f("x")[:T]
    W_ih0, W_hh0 = f("W_ih0"), f("W_hh0")
    W_ih1, W_hh1 = f("W_ih1"), f("W_hh1")
    W_xi = f("W_xi")
    W_out, W_fc = f("W_out"), f("W_fc")
    for k in ("b_ih0", "b_hh0", "b_ih1", "b_hh1", "b_xi", "b_out", "b_fc"):
        assert np.abs(f(k)).max() == 0.0, f"kernel assumes {k} == 0"

    gs = np.repeat(np.array([-1.0, -1.0, 2.0, -1.0], np.float32), H)  # (512,)

    xb = (x @ W_ih0[:, :X_DIM].T) * gs[None, :]
    # [128, T//U, U*4]: chunk-contiguous per partition, (u, g) order
    xb3 = np.ascontiguousarray(
        xb.reshape(T // U, U, 4, H).transpose(3, 0, 1, 2)
        .reshape(H, T // U, 4 * U))

    w0ht = np.ascontiguousarray((W_hh0 * gs[:, None]).T)              # 128x512
    w0rmt = np.ascontiguousarray((W_ih0[:, X_DIM:] * gs[:, None]).T)  # 20x512
    w1it = np.ascontiguousarray((W_ih1 * gs[:, None]).T)
    w1ht = np.ascontiguousarray((W_hh1 * gs[:, None]).T)

    # interface: cols [wv(20) tanh | er(20) sig | fg ga gw sig | rb wb sp |
    #                  pi(3) raw]
    idx = np.concatenate([
        np.arange(62, 82),        # wv
        np.arange(42, 62),        # er
        [82, 83, 84],             # fg ga gw
        [20, 41],                 # rb wb
        [85, 86, 87],             # pi
        [82] * 10,                # fg x10 (free-dim broadcast)
    ]).astype(np.int64)
    sc = np.concatenate([
        2.0 * np.ones(20), -1.0 * np.ones(20), -1.0 * np.ones(3),
        np.ones(2), np.ones(3), -1.0 * np.ones(10),
    ]).astype(np.float32)
    wxit = np.ascontiguousarray((W_xi[idx] * sc[:, None]).T)          # 128x48
    wrkt = np.ascontiguousarray((W_xi[0:20] * 2.0).T)                 # 128x20
    wwkt = np.ascontiguousarray((W_xi[21:41] * 2.0).T)                # 128x20

    Wcomb = W_fc @ W_out                                              # 10x148
    wct1 = np.ascontiguousarray(Wcomb[:, :H].T)                       # 128x10
    wct2 = np.ascontiguousarray(Wcomb[:, H:].T)                       # 20x10

    jlt = np.tril(np.ones((10, 10), np.float32), -1)
    diagm = (1.0 - np.eye(10)).astype(np.float32)

    f16 = np.float16
    return {
        "xb3": xb3, "w0ht": w0ht.astype(f16), "w0rmt": w0rmt,
        "w1it": w1it.astype(f16), "w1ht": w1ht.astype(f16),
        "wxit": wxit.astype(f16), "wrkt": wrkt.astype(f16),
        "wwkt": wwkt.astype(f16), "wct1": wct1.astype(f16),
        "wct2": wct2, "jlt": jlt, "diagm": diagm,
    }


_BUILD_CACHE = {}


def kernel(**inputs):
    T = np.asarray(inputs["x"]).shape[0]
    U = 8
    key = (T, U)
    if key not in _BUILD_CACHE:
        _BUILD_CACHE[key] = build(T=T, U=U)
    nc, names = _BUILD_CACHE[key]
    logical = prep_inputs(inputs, T=T, U=U)
    in_map = {names[k]: v for k, v in logical.items()}

    from concourse.bass_utils import run_bass_kernel_spmd
    res = run_bass_kernel_spmd(nc, [in_map], core_ids=[0])
    yt = res.results[0][names["yt"]]          # (10, T//U, U)
    y = np.asarray(yt).reshape(10, T)
    return np.ascontiguousarray(y.T)[None].astype(np.float32)


# revision 6
# speedup vs baseline: 2.2358x; 1.2091x over previous
"""DNC sequential kernel v2 for Trainium2 — row-major redesign.

Single-core Bass/Tile implementation of the 8192-step DNC recurrence.
Design vs the v1 baseline:
- Replicated-output PE matmuls give the interface activations pre-broadcast
  over the 10 memory cells; no gpsimd partition_broadcast anywhere.
- stream_shuffle (DVE) does the remaining partition broadcasts.
- All dot products via scalar_tensor_tensor + accum_out (1 DVE op) or tiny
  PE matmuls in row form.
- Sort-free allocation: pairwise-compare matrix + product-reduce (DVE
  tensor_reduce with mult), stable-argsort tie semantics via is_equal*JLT.
- Read vector rv is never materialized: gates0 use W0rM = M @ W_ih0[:,10:]^T
  (f32r matmul, precomputed per step once M is updated) applied to rw, and
  the output projection uses WcM = M @ Wc2^T applied to rw.
- ACT table set 6 (exp/ln) preloaded once; sigmoid/tanh via exp with signs
  folded into host weights; softplus = Ln(1+e^x) exact; sqrt via Ln+Exp(0.5).
- All biases in this problem are zero (asserted on host) and dropped.
- Controller-side matmuls (gates, interface, output proj) use fp16
  stationaries + fp16 hidden states: fp32 [.,128] weight loads cost ~800ns
  on the PE, fp16 loads ~4x less; all DNC memory-state math stays fp32.
- Input xb3 is chunk-contiguous [128, T/U, 4U] so each chunk DMA is 128
  large descriptors instead of 512 16-byte ones; output DMA issues from
  the gpsimd queue so input prefetch on SP runs a chunk ahead.
"""

import sys
import os
import numpy as np

if "/opt/trn_rl_repo" not in sys.path:
    sys.path.insert(0, "/opt/trn_rl_repo")

N, CELL, R, H, X_DIM, OUT_DIM = 10, 20, 1, 128, 10, 10
EPS, DELTA = 1e-6, 1e-6
SEQ_LEN = 8192


def build(T=SEQ_LEN, U=8):
    import concourse.bass as bass
    import concourse.bacc as bacc
    import concourse.mybir as mybir
    from concourse import tile

    F32 = mybir.dt.float32
    F32R = mybir.dt.float32r
    F16 = mybir.dt.float16
    AF = mybir.ActivationFunctionType
    OP = mybir.AluOpType
    AX = mybir.AxisListType
    ds = bass.ds

    assert T % U == 0

    nc = bacc.Bacc(None, target_bir_lowering=False, debug=False)
    names = {}

    with tile.TileContext(nc) as tc:
        with tc.tile_pool(name="dram", bufs=1, space="DRAM") as dram, \
             tc.tile_pool(name="consts", bufs=1) as consts, \
             tc.tile_pool(name="state", bufs=1) as state, \
             tc.tile_pool(name="work", bufs=2) as work, \
             tc.tile_pool(name="io", bufs=2) as io, \
             tc.tile_pool(name="psG", bufs=2, space="PSUM") as psG, \
             tc.tile_pool(name="psA", bufs=2, space="PSUM") as psA, \
             tc.tile_pool(name="psB", bufs=2, space="PSUM") as psB, \
             tc.tile_pool(name="psW", bufs=1, space="PSUM") as psW:

            # ---------------- DRAM I/O ----------------
            d_xb3 = dram.tile([128, T // U, 4 * U], F32, kind="ExternalInput")
            d_w0ht = dram.tile([128, 512], F16, kind="ExternalInput")
            d_w0rmt = dram.tile([20, 512], F32R, kind="ExternalInput")
            d_w1it = dram.tile([128, 512], F16, kind="ExternalInput")
            d_w1ht = dram.tile([128, 512], F16, kind="ExternalInput")
            d_wxit = dram.tile([128, 58], F16, kind="ExternalInput")
            d_wrkt = dram.tile([128, 20], F16, kind="ExternalInput")
            d_wwkt = dram.tile([128, 20], F16, kind="ExternalInput")
            d_wct1 = dram.tile([128, 10], F16, kind="ExternalInput")
            d_wct2 = dram.tile([20, 10], F32, kind="ExternalInput")
            d_jlt = dram.tile([10, 10], F32, kind="ExternalInput")
            d_diagm = dram.tile([10, 10], F32, kind="ExternalInput")
            d_yt = dram.tile([10, T // U, U], F32, kind="ExternalOutput")
            for k, v in [("xb3", d_xb3), ("w0ht", d_w0ht), ("w0rmt", d_w0rmt),
                         ("w1it", d_w1it), ("w1ht", d_w1ht), ("wxit", d_wxit),
                         ("wrkt", d_wrkt), ("wwkt", d_wwkt), ("wct1", d_wct1),
                         ("wct2", d_wct2),
                         ("jlt", d_jlt), ("diagm", d_diagm),
                         ("yt", d_yt)]:
                names[k] = v.tensor.name

            # preload ACT set 6 (exp/ln) once; all activations stay in-set
            nc.scalar.add_instruction(mybir.InstLoadActFuncSet(
                name=nc.get_next_instruction_name(), act_func_set_id=6,
                ins=[], outs=[]))

            # ---------------- const SBUF ----------------
            W0HT = consts.tile([128, 512], F16)
            W0RMT = consts.tile([20, 512], F32R)
            W1IT = consts.tile([128, 512], F16)
            W1HT = consts.tile([128, 512], F16)
            WXIT = consts.tile([128, 58], F16)
            WRKT = consts.tile([128, 20], F16)
            WWKT = consts.tile([128, 20], F16)
            WCT1 = consts.tile([128, 10], F16)
            WCT2 = consts.tile([20, 10], F32)
            JLT = consts.tile([10, 10], F32)
            DIAGM = consts.tile([10, 10], F32)
            E38 = consts.tile([1, 10], F32)
            ONESC = consts.tile([10, 1], F32)
            ONES10 = consts.tile([128, 10], F32)

            for dst, src in [(W0HT, d_w0ht), (W0RMT, d_w0rmt), (W1IT, d_w1it),
                             (W1HT, d_w1ht), (WXIT, d_wxit), (WRKT, d_wrkt),
                             (WWKT, d_wwkt), (WCT1, d_wct1), (WCT2, d_wct2),
                             (JLT, d_jlt), (DIAGM, d_diagm)]:
                nc.sync.dma_start(dst[:], src[:])
            nc.vector.memset(E38[:], 1e-36)
            nc.vector.memset(ONESC[:], 1.0)
            nc.vector.memset(ONES10[:], 1.0)

            # ---------------- state SBUF ----------------
            h0 = state.tile([128, 1], F16)
            c0 = state.tile([128, 1], F32)
            h1rep = state.tile([128, 10], F16)   # h1 replicated; col 0 = h1
            c1 = state.tile([128, 1], F32)
            M32 = state.tile([32, 32], F32)      # M  = [0:10, 0:20]
            Mt32 = state.tile([32, 32], F32)     # Mt = [0:20, 0:10]
            L32 = state.tile([32, 32], F32)      # L  = [0:10, 0:10]
            LT32 = state.tile([32, 32], F32)
            Mtr = state.tile([20, 10], F32R)     # f32r copy of Mt
            zM32 = state.tile([32, 32], F32)     # col 0 = zM (row norms^2)
            zMr32 = state.tile([32, 32], F32)    # row 0 = zM as a row
            RW32 = state.tile([32, 32], F32)     # row 0 = rw_row
            RWT32 = state.tile([32, 32], F32)    # col 0 = rw_col
            WW32 = state.tile([32, 32], F32)     # row 0 = ww_row
            WWT32 = state.tile([32, 32], F32)    # col 0 = ww_col
            Psh = state.tile([32, 32], F32)      # row 0 = p_row
            P10b = state.tile([32, 10], F32)     # p broadcast
            p10m = state.tile([10, 10], F32)     # p bcast, diag-masked
            UEC32 = state.tile([32, 32], F32)    # col0 = ue
            UET32 = state.tile([32, 32], F32)    # row0 = ue
            UE10 = state.tile([32, 10], F32)     # ue bcast over partitions
            W0RM = state.tile([10, 512], F16)    # M @ W0r^T (folded)
            WCM = state.tile([10, 10], F16)      # M @ Wc2^T
            scr = state.tile([10, 20], F32)      # accum dst scratch
            rwc16 = state.tile([10, 1], F16)     # f16 copy of rw_col

            for t in (h0, c0, h1rep, c1, M32, Mt32, L32, LT32, zM32, zMr32,
                      RW32, RWT32, WW32, WWT32, Psh, P10b, p10m, UEC32,
                      UET32, UE10, W0RM, WCM, scr):
                nc.vector.memset(t[:], 0.0)
            # ue state must start at DELTA (u=0 <-> ue=delta)
            nc.vector.memset(UEC32[0:10, 0:1], DELTA)


            M = M32[0:10, 0:20]
            Mt = Mt32[0:20, 0:10]
            L = L32[0:10, 0:10]
            LT = LT32[0:10, 0:10]
            zMc = zM32[0:10, 0:1]
            zMr = zMr32[0:1, 0:10]
            rw_row = RW32[0:1, 0:10]
            rw_col = RWT32[0:10, 0:1]
            ww_row = WW32[0:1, 0:10]
            ww_col = WWT32[0:10, 0:1]
            p_row = Psh[0:1, 0:10]
            ue_col = UEC32[0:10, 0:1]

            MASK0 = [0] * 32

            def lstm_cell(G, E_t, c, hout, tag, xb_ap=None):
                """PSUM gates (i,f,g,o sign-folded) -> c/h update; hout is
                [128,1] (cell0) or writes h1rep (cell1 handled by caller)."""
                if xb_ap is not None:
                    z = work.tile([128, 4], F32, tag=f"z{tag}")
                    nc.vector.tensor_tensor(out=z[:], in0=G[:], in1=xb_ap,
                                            op=OP.add)
                    src = z[:]
                else:
                    src = G[:]
                E = work.tile([128, 4], F32, tag=f"E{tag}")
                nc.scalar.activation(E[:], src, AF.Exp)
                D = work.tile([128, 4], F32, tag=f"D{tag}")
                nc.scalar.add(D[:], E[:], 1.0)
                Rr = work.tile([128, 4], F32, tag=f"R{tag}")
                nc.vector.reciprocal(Rr[:], D[:])
                m_ = work.tile([128, 1], F32, tag=f"m{tag}")
                nc.vector.tensor_tensor(out=m_[:], in0=Rr[:, 0:1],
                                        in1=Rr[:, 2:3], op=OP.mult)
                A_ = work.tile([128, 1], F32, tag=f"A{tag}")
                nc.vector.scalar_tensor_tensor(
                    out=A_[:], in0=m_[:], scalar=-2.0, in1=Rr[:, 0:1],
                    op0=OP.mult, op1=OP.add)
                nc.vector.scalar_tensor_tensor(
                    out=c[:], in0=c[:], scalar=Rr[:, 1:2], in1=A_[:],
                    op0=OP.mult, op1=OP.add)
                E2 = work.tile([128, 1], F32, tag=f"E2{tag}")
                nc.scalar.activation(E2[:], c[:], AF.Exp, scale=2.0)
                D2 = work.tile([128, 1], F32, tag=f"D2{tag}")
                nc.scalar.add(D2[:], E2[:], 1.0)
                R2 = work.tile([128, 1], F32, tag=f"R2{tag}")
                nc.vector.reciprocal(R2[:], D2[:])
                m2 = work.tile([128, 1], F32, tag=f"m2{tag}")
                nc.vector.tensor_tensor(out=m2[:], in0=Rr[:, 3:4], in1=R2[:],
                                        op=OP.mult)
                return Rr, m2

            SKIP_W0RM = os.environ.get("SKIP_W0RM", "0") == "1"
            SKIP_MEM = os.environ.get("SKIP_MEM", "0") == "1"
            SKIP_CTRL = os.environ.get("SKIP_CTRL", "0") == "1"

            def step(XB, YT, u):
                # ======== controller cell 0 ========
                if SKIP_CTRL:
                    nc.vector.tensor_copy(YT[:, u:u + 1], XB[0:10, u, 0:1])
                    return
                GG = psG.tile([128, 8], F32, tag="GG")
                G0 = GG[:, 0:4]
                for g in range(4):
                    nc.tensor.matmul(G0[:, g:g + 1],
                                     lhsT=W0HT[:, 128 * g:128 * (g + 1)],
                                     rhs=h0[:], start=True, stop=False)
                    nc.tensor.matmul(G0[:, g:g + 1],
                                     lhsT=W0RM[0:10, 128 * g:128 * (g + 1)],
                                     rhs=rwc16[:], start=False, stop=True)
                Rr0, m20 = lstm_cell(G0, None, c0, h0, "0",
                                     xb_ap=XB[:, 4 * u:4 * u + 4])
                nc.vector.scalar_tensor_tensor(
                    out=h0[:], in0=m20[:], scalar=-2.0, in1=Rr0[:, 3:4],
                    op0=OP.mult, op1=OP.add)

                # ======== controller cell 1 ========
                G1 = GG[:, 4:8]
                for g in range(4):
                    nc.tensor.matmul(G1[:, g:g + 1],
                                     lhsT=W1IT[:, 128 * g:128 * (g + 1)],
                                     rhs=h0[:], start=True, stop=False)
                    nc.tensor.matmul(G1[:, g:g + 1],
                                     lhsT=W1HT[:, 128 * g:128 * (g + 1)],
                                     rhs=h1rep[:, 0:1], start=False, stop=True)
                Rr1, m21 = lstm_cell(G1, None, c1, None, "1")
                th1 = work.tile([128, 1], F32, tag="th1")
                nc.vector.tensor_scalar(out=th1[:], in0=m21[:], scalar1=-2.0,
                                        scalar2=None, op0=OP.mult)
                # h1 replicated x10: (ones*th1)+Ro  per partition
                nc.vector.tensor_scalar(out=h1rep[:], in0=ONES10[:],
                                        scalar1=th1[:], scalar2=Rr1[:, 3:4],
                                        op0=OP.mult, op1=OP.add)

                # ======== interface ========
                XIPS = psA.tile([32, 80], F32, tag="XIPS")
                nc.tensor.matmul(XIPS[0:10, 0:58], lhsT=h1rep[:],
                                 rhs=WXIT[:], start=True, stop=True)
                nc.tensor.matmul(XIPS[0:20, 64:65], lhsT=WRKT[:],
                                 rhs=h1rep[:, 0:1], start=True, stop=True)
                nc.tensor.matmul(XIPS[0:20, 65:66], lhsT=WWKT[:],
                                 rhs=h1rep[:, 0:1], start=True, stop=True)
                EX = work.tile([10, 58], F32, tag="EX")
                nc.scalar.activation(EX[:], XIPS[0:10, 0:58], AF.Exp)
                EK = work.tile([20, 2], F32, tag="EK")
                nc.scalar.activation(EK[:], XIPS[0:20, 64:66], AF.Exp)
                DX = work.tile([10, 58], F32, tag="DX")
                nc.scalar.add(DX[:], EX[:], 1.0)
                RX = work.tile([10, 58], F32, tag="RX")
                nc.vector.reciprocal(RX[:], DX[:])
                tauwv = work.tile([10, 20], F32, tag="tauwv")
                nc.vector.tensor_scalar(out=tauwv[:], in0=RX[:, 0:20],
                                        scalar1=-2.0, scalar2=1.0,
                                        op0=OP.mult, op1=OP.add)
                DK = work.tile([20, 2], F32, tag="DK")
                nc.scalar.add(DK[:], EK[:], 1.0)
                RK = work.tile([20, 2], F32, tag="RK")
                nc.vector.reciprocal(RK[:], DK[:])
                KT = work.tile([20, 2], F32, tag="KT")
                nc.vector.tensor_scalar(out=KT[:], in0=RK[:], scalar1=-2.0,
                                        scalar2=1.0, op0=OP.mult, op1=OP.add)
                # softplus strengths: Ln(1+e^x); col43=rb col44=wb
                BS = work.tile([1, 2], F32, tag="BS")
                nc.scalar.activation(BS[:], DX[0:1, 43:45], AF.Ln)
                # pi softmax pieces (cols 45:48 of EX are exp(pre) already)
                Spi = work.tile([1, 1], F32, tag="Spi")
                nc.vector.tensor_reduce(Spi[:], EX[0:1, 45:48],
                                        axis=AX.X, op=OP.add)
                rSpi = work.tile([1, 1], F32, tag="rSpi")
                nc.vector.reciprocal(rSpi[:], Spi[:])
                qv = work.tile([1, 3], F32, tag="qv")
                nc.vector.tensor_scalar(out=qv[:], in0=EX[0:1, 45:48],
                                        scalar1=rSpi[:], scalar2=None,
                                        op0=OP.mult)
                # early small combines: mga = gw*ga ; c2 = gw - mga
                mga = work.tile([1, 1], F32, tag="mga")
                nc.vector.tensor_tensor(out=mga[:], in0=RX[0:1, 42:43],
                                        in1=RX[0:1, 41:42], op=OP.mult)
                c2 = work.tile([1, 1], F32, tag="c2")
                nc.vector.tensor_tensor(out=c2[:], in0=RX[0:1, 42:43],
                                        in1=mga[:], op=OP.subtract)

                if SKIP_MEM:
                    nc.vector.tensor_copy(YT[:, u:u + 1], RX[0:10, 0:1])
                    return
                # ======== content-write dots on OLD memory (PE) ========
                PSB = psB.tile([32, 64], F32, tag="PSB")
                DOTW = PSB
                nc.tensor.matmul(DOTW[0:1, 0:10], lhsT=KT[:, 1:2],
                                 rhs=Mt, start=True, stop=True)
                nc.tensor.matmul(DOTW[0:1, 10:11], lhsT=KT[:, 1:2],
                                 rhs=KT[:, 1:2], start=True, stop=True)
                nc.tensor.matmul(DOTW[0:1, 11:12], lhsT=KT[:, 0:1],
                                 rhs=KT[:, 0:1], start=True, stop=True)

                # ======== usage + allocation (DVE) ========
                fg = RX[:, 40:41]                 # [10,1] replicated
                # ue' = DELTA + [(ue-DELTA) + ww(1-ue)] * (1 - fg*rw)
                a_ = work.tile([10, 1], F32, tag="a_")
                nc.vector.tensor_scalar(out=a_[:], in0=ue_col, scalar1=-1.0,
                                        scalar2=1.0, op0=OP.mult, op1=OP.add)
                b_ = work.tile([10, 1], F32, tag="b_")
                nc.vector.scalar_tensor_tensor(
                    out=b_[:], in0=a_[:], scalar=ww_col, in1=ue_col,
                    op0=OP.mult, op1=OP.add)      # ue + ww(1-ue)
                bd = work.tile([10, 1], F32, tag="bd")
                nc.vector.tensor_scalar(out=bd[:], in0=b_[:], scalar1=1.0,
                                        scalar2=-DELTA, op0=OP.mult,
                                        op1=OP.add)
                npsi = work.tile([10, 1], F32, tag="npsi")
                nc.vector.scalar_tensor_tensor(
                    out=npsi[:], in0=RWT32[0:10, 0:1], scalar=fg,
                    in1=ONESC[:], op0=OP.mult, op1=OP.subtract)  # fg*rw - 1
                tue = work.tile([10, 1], F32, tag="tue")
                nc.vector.scalar_tensor_tensor(
                    out=tue[:], in0=bd[:], scalar=-1.0, in1=npsi[:],
                    op0=OP.mult, op1=OP.mult)     # bd * psi
                nc.vector.tensor_scalar(out=ue_col, in0=tue[:], scalar1=1.0,
                                        scalar2=DELTA, op0=OP.mult,
                                        op1=OP.add)
                nc.vector.transpose(UET32[:], UEC32[:])
                nc.vector.stream_shuffle(UE10[:], UET32[0:32, 0:10], MASK0)
                eqJ = work.tile([10, 10], F32, tag="eqJ")
                nc.vector.scalar_tensor_tensor(
                    out=eqJ[:], in0=UE10[0:10, :], scalar=ue_col,
                    in1=JLT[:], op0=OP.is_equal, op1=OP.mult)
                cmp = work.tile([10, 10], F32, tag="cmp")
                nc.vector.scalar_tensor_tensor(
                    out=cmp[:], in0=UE10[0:10, :], scalar=ue_col,
                    in1=eqJ[:], op0=OP.is_lt, op1=OP.add)
                tm = work.tile([10, 10], F32, tag="tm")
                nc.vector.tensor_scalar(out=tm[:], in0=UE10[0:10, :],
                                        scalar1=1.0, scalar2=-1.0,
                                        op0=OP.mult, op1=OP.add)  # ue-1
                sel = work.tile([10, 10], F32, tag="sel")
                nc.vector.tensor_tensor(out=sel[:], in0=cmp[:], in1=tm[:],
                                        op=OP.mult)
                nc.vector.tensor_scalar(out=sel[:], in0=sel[:], scalar1=1.0,
                                        scalar2=None, op0=OP.add)
                prod = work.tile([10, 1], F32, tag="prod")
                nc.vector.tensor_reduce(prod[:], sel[:], axis=AX.X,
                                        op=OP.mult)
                s1 = work.tile([10, 1], F32, tag="s1")
                nc.vector.tensor_scalar(out=s1[:], in0=ue_col, scalar1=-1.0,
                                        scalar2=1.0, op0=OP.mult, op1=OP.add)
                AL32 = work.tile([32, 32], F32, tag="AL32")
                nc.vector.tensor_tensor(out=AL32[0:10, 0:1], in0=s1[:],
                                        in1=prod[:], op=OP.mult)
                ALT = work.tile([32, 32], F32, tag="ALT")
                nc.vector.transpose(ALT[:], AL32[:])

                # ======== content write weights (row form) ========
                q2w = work.tile([1, 10], F32, tag="q2w")
                nc.vector.scalar_tensor_tensor(
                    out=q2w[:], in0=zMr, scalar=DOTW[0:1, 10:11],
                    in1=E38[:], op0=OP.mult, op1=OP.max)
                SQW = work.tile([1, 10], F32, tag="SQW")
                nc.scalar.activation(SQW[:], q2w[:], AF.Ln)
                nc.scalar.activation(SQW[:], SQW[:], AF.Exp, scale=0.5)
                dw = work.tile([1, 10], F32, tag="dw")
                nc.vector.tensor_scalar(out=dw[:], in0=SQW[:], scalar1=EPS,
                                        scalar2=None, op0=OP.add)
                rdw = work.tile([1, 10], F32, tag="rdw")
                nc.vector.reciprocal(rdw[:], dw[:])
                simw = work.tile([1, 10], F32, tag="simw")
                nc.vector.tensor_tensor(out=simw[:], in0=DOTW[0:1, 0:10],
                                        in1=rdw[:], op=OP.mult)
                EW = work.tile([1, 10], F32, tag="EW")
                Sw = work.tile([1, 1], F32, tag="Sw")
                nc.scalar.activation(EW[:], simw[:], AF.Exp,
                                     scale=BS[0:1, 1:2], accum_out=Sw[:])
                rSw = work.tile([1, 1], F32, tag="rSw")
                nc.vector.reciprocal(rSw[:], Sw[:])
                te = work.tile([1, 10], F32, tag="te")
                nc.vector.tensor_scalar(out=te[:], in0=EW[:], scalar1=c2[:],
                                        scalar2=rSw[:], op0=OP.mult,
                                        op1=OP.mult)
                sw = work.tile([1, 1], F32, tag="sw")
                nc.vector.scalar_tensor_tensor(
                    out=ww_row, in0=ALT[0:1, 0:10], scalar=mga[:], in1=te[:],
                    op0=OP.mult, op1=OP.add, accum_out=sw[:])
                WW10 = work.tile([32, 10], F32, tag="WW10")
                nc.vector.stream_shuffle(WW10[:], WW32[0:32, 0:10], MASK0)
                nc.vector.transpose(WWT32[:], WW32[:])

                # ======== M update ========
                m1 = work.tile([10, 20], F32, tag="m1")
                nc.vector.scalar_tensor_tensor(
                    out=m1[:], in0=RX[:, 20:40], scalar=ww_col, in1=M,
                    op0=OP.mult, op1=OP.mult)
                M2t = work.tile([10, 20], F32, tag="M2t")
                nc.vector.tensor_tensor(out=M2t[:], in0=M, in1=m1[:],
                                        op=OP.subtract)
                nc.vector.scalar_tensor_tensor(
                    out=M, in0=tauwv[:], scalar=ww_col, in1=M2t[:],
                    op0=OP.mult, op1=OP.add)
                nc.vector.scalar_tensor_tensor(
                    out=scr[:], in0=M, scalar=1.0, in1=M,
                    op0=OP.mult, op1=OP.mult, accum_out=zMc)
                nc.vector.transpose(Mt32[:], M32[:])
                nc.vector.transpose(zMr32[:], zM32[:])
                nc.vector.tensor_copy(Mtr[:], Mt)
                if not SKIP_W0RM:
                    W0RMps = psW.tile([10, 512], F32, tag="W0RMps")
                    nc.tensor.matmul(W0RMps[:], lhsT=Mtr[:], rhs=W0RMT[:],
                                     start=True, stop=True)
                    nc.scalar.copy(W0RM[:], W0RMps[:])

                # ======== L update ========
                lt1 = work.tile([10, 10], F32, tag="lt1")
                nc.vector.scalar_tensor_tensor(
                    out=lt1[:], in0=WW10[0:10, :], scalar=ww_col, in1=L,
                    op0=OP.add, op1=OP.mult)
                lt2 = work.tile([10, 10], F32, tag="lt2")
                nc.vector.tensor_tensor(out=lt2[:], in0=L, in1=lt1[:],
                                        op=OP.subtract)
                nc.vector.scalar_tensor_tensor(
                    out=L, in0=p10m[:], scalar=ww_col, in1=lt2[:],
                    op0=OP.mult, op1=OP.add)
                nc.vector.transpose(LT32[:], L32[:])
                tp = work.tile([1, 10], F32, tag="tp")
                nc.vector.scalar_tensor_tensor(
                    out=tp[:], in0=p_row, scalar=sw[:], in1=ww_row,
                    op0=OP.mult, op1=OP.subtract)
                nc.vector.tensor_tensor(out=p_row, in0=p_row, in1=tp[:],
                                        op=OP.subtract)
                nc.vector.stream_shuffle(P10b[:], Psh[0:32, 0:10], MASK0)
                nc.gpsimd.tensor_tensor(out=p10m[:], in0=P10b[0:10, :],
                                        in1=DIAGM[:], op=OP.mult)

                # ======== read dots (PE, new memory) ========
                nc.tensor.matmul(PSB[0:1, 16:26], lhsT=rw_col, rhs=L,
                                 start=True, stop=True)          # bwd
                nc.tensor.matmul(PSB[0:1, 26:36], lhsT=rw_col, rhs=LT,
                                 start=True, stop=True)          # fwd
                nc.tensor.matmul(PSB[0:1, 36:46], lhsT=KT[:, 0:1], rhs=Mt,
                                 start=True, stop=True)          # dotr

                # ======== content read weights ========
                q2r = work.tile([1, 10], F32, tag="q2r")
                nc.vector.scalar_tensor_tensor(
                    out=q2r[:], in0=zMr, scalar=DOTW[0:1, 11:12],
                    in1=E38[:], op0=OP.mult, op1=OP.max)
                SQR = work.tile([1, 10], F32, tag="SQR")
                nc.scalar.activation(SQR[:], q2r[:], AF.Ln)
                nc.scalar.activation(SQR[:], SQR[:], AF.Exp, scale=0.5)
                dr = work.tile([1, 10], F32, tag="dr")
                nc.vector.tensor_scalar(out=dr[:], in0=SQR[:], scalar1=EPS,
                                        scalar2=None, op0=OP.add)
                rdr = work.tile([1, 10], F32, tag="rdr")
                nc.vector.reciprocal(rdr[:], dr[:])
                simr = work.tile([1, 10], F32, tag="simr")
                nc.vector.tensor_tensor(out=simr[:], in0=PSB[0:1, 36:46],
                                        in1=rdr[:], op=OP.mult)
                ER = work.tile([1, 10], F32, tag="ER")
                Sr = work.tile([1, 1], F32, tag="Sr")
                nc.scalar.activation(ER[:], simr[:], AF.Exp,
                                     scale=BS[0:1, 0:1], accum_out=Sr[:])
                rSr = work.tile([1, 1], F32, tag="rSr")
                nc.vector.reciprocal(rSr[:], Sr[:])
                q1p = work.tile([1, 1], F32, tag="q1p")
                nc.vector.tensor_tensor(out=q1p[:], in0=qv[0:1, 1:2],
                                        in1=rSr[:], op=OP.mult)
                ra = work.tile([1, 10], F32, tag="ra")
                nc.vector.tensor_scalar(out=ra[:], in0=PSB[0:1, 16:26],
                                        scalar1=qv[0:1, 0:1], scalar2=None,
                                        op0=OP.mult)
                rb_ = work.tile([1, 10], F32, tag="rb_")
                nc.vector.scalar_tensor_tensor(
                    out=rb_[:], in0=PSB[0:1, 26:36], scalar=qv[0:1, 2:3],
                    in1=ra[:], op0=OP.mult, op1=OP.add)
                nc.vector.scalar_tensor_tensor(
                    out=rw_row, in0=ER[:], scalar=q1p[:], in1=rb_[:],
                    op0=OP.mult, op1=OP.add)
                nc.vector.transpose(RWT32[:], RW32[:])
                nc.scalar.copy(rwc16[:], RWT32[0:10, 0:1])

                # ======== output y ========
                nc.tensor.matmul(PSB[0:10, 46:56], lhsT=Mt, rhs=WCT2[:],
                                 start=True, stop=True)
                nc.scalar.copy(WCM[:], PSB[0:10, 46:56])
                nc.tensor.matmul(PSB[0:10, 60:61], lhsT=WCM[:], rhs=rwc16[:],
                                 start=True, stop=False)
                nc.tensor.matmul(PSB[0:10, 60:61], lhsT=WCT1[:],
                                 rhs=h1rep[:, 0:1], start=False, stop=True)
                nc.scalar.copy(YT[:, u:u + 1], PSB[0:10, 60:61])



            # ================= main loop =================
            import concourse.mybir as _mb
            _stag = os.environ.get("STAG", "1") == "1"
            _hints = () if os.environ.get("NO_HINTS", "0") == "1" else tuple(_mb.ALL_ENGINES)
            with tc.For_i(0, T // U, 1, hint_engines=_hints,
                          staggered_reset=_stag) as iv:
                XB = io.tile([128, 4 * U], F32, tag="XB")
                nc.sync.dma_start(XB[:], d_xb3[:, ds(iv, 1), :])
                YT = io.tile([10, U], F32, tag="YT")
                for u in range(U):
                    step(XB, YT, u)
                nc.gpsimd.dma_start(d_yt[:, ds(iv, 1), :], YT[:])

    nc.compile()
    return nc, names


def prep_inputs(inputs, T=SEQ_LEN, U=8):
    f = lambda k: np.asarray(inputs[k], dtype=np.float32)
    x = f("x")[:T]
    W_ih0, W_hh0 = f("W_ih0"), f("W_hh0")
    W_ih1, W_hh1 = f("W_ih1"), f("W_hh1")
    W_xi = f("W_xi")
    W_out, W_fc = f("W_out"), f("W_fc")
    for k in ("b_ih0", "b_hh0", "b_ih1", "b_hh1", "b_xi", "b_out", "b_fc"):
        assert np.abs(f(k)).max() == 0.0, f"kernel assumes {k} == 0"

    gs = np.repeat(np.array([-1.0, -1.0, 2.0, -1.0], np.float32), H)  # (512,)

    xb = (x @ W_ih0[:, :X_DIM].T) * gs[None, :]
    # [128, T//U, U*4]: chunk-contiguous per partition, (u, g) order
    xb3 = np.ascontiguousarray(
        xb.reshape(T // U, U, 4, H).transpose(3, 0, 1, 2)
        .reshape(H, T // U, 4 * U))

    w0ht = np.ascontiguousarray((W_hh0 * gs[:, None]).T)              # 128x512
    w0rmt = np.ascontiguousarray((W_ih0[:, X_DIM:] * gs[:, None]).T)  # 20x512
    w1it = np.ascontiguousarray((W_ih1 * gs[:, None]).T)
    w1ht = np.ascontiguousarray((W_hh1 * gs[:, None]).T)

    # interface: cols [wv(20) tanh | er(20) sig | fg ga gw sig | rb wb sp |
    #                  pi(3) raw]
    idx = np.concatenate([
        np.arange(62, 82),        # wv
        np.arange(42, 62),        # er
        [82, 83, 84],             # fg ga gw
        [20, 41],                 # rb wb
        [85, 86, 87],             # pi
        [82] * 10,                # fg x10 (free-dim broadcast)
    ]).astype(np.int64)
    sc = np.concatenate([
        2.0 * np.ones(20), -1.0 * np.ones(20), -1.0 * np.ones(3),
        np.ones(2), np.ones(3), -1.0 * np.ones(10),
    ]).astype(np.float32)
    wxit = np.ascontiguousarray((W_xi[idx] * sc[:, None]).T)          # 128x48
    wrkt = np.ascontiguousarray((W_xi[0:20] * 2.0).T)                 # 128x20
    wwkt = np.ascontiguousarray((W_xi[21:41] * 2.0).T)                # 128x20

    Wcomb = W_fc @ W_out                                              # 10x148
    wct1 = np.ascontiguousarray(Wcomb[:, :H].T)                       # 128x10
    wct2 = np.ascontiguousarray(Wcomb[:, H:].T)                       # 20x10

    jlt = np.tril(np.ones((10, 10), np.float32), -1)
    diagm = (1.0 - np.eye(10)).astype(np.float32)

    f16 = np.float16
    return {
        "xb3": xb3, "w0ht": w0ht.astype(f16), "w0rmt": w0rmt,
        "w1it": w1it.astype(f16), "w1ht": w1ht.astype(f16),
        "wxit": wxit.astype(f16), "wrkt": wrkt.astype(f16),
        "wwkt": wwkt.astype(f16), "wct1": wct1.astype(f16),
        "wct2": wct2, "jlt": jlt, "diagm": diagm,
    }


_BUILD_CACHE = {}


def kernel(**inputs):
    T = np.asarray(inputs["x"]).shape[0]
    U = 8
    key = (T, U)
    if key not in _BUILD_CACHE:
        _BUILD_CACHE[key] = build(T=T, U=U)
    nc, names = _BUILD_CACHE[key]
    logical = prep_inputs(inputs, T=T, U=U)
    in_map = {names[k]: v for k, v in logical.items()}

    from concourse.bass_utils import run_bass_kernel_spmd
    res = run_bass_kernel_spmd(nc, [in_map], core_ids=[0])
    yt = res.results[0][names["yt"]]          # (10, T//U, U)
    y = np.asarray(yt).reshape(10, T)
    return np.ascontiguousarray(y.T)[None].astype(np.float32)
